# revision 26
# baseline (speedup 1.0000x reference)
"""BeitSelfAttention block-sparse attention kernel for 8 Trainium2 NeuronCores.

Strategy (data-parallel over batch, B=8 -> one batch element per core):
  - fp8e4 DoubleRow matmuls for QKV projections and block-sparse scores
    (two 128-row k-tiles per pass at 0.5 cycles/row).
  - The relative-position bias AND the block-sparsity mask are host-packed
    into one fp8 table (16x scale) and added into the score PSUM by an
    identity DoubleRow matmul; gather multiplicity (rand/local block
    collisions) is realized by a few extra AV matmuls against half-masked V
    copies, so no per-element multiply is needed on DVE at all.
  - The cls KEY rides as a 99th score row per key-pair tile (designated to
    one pair per query via the bias mask); the cls QUERY is packed column 0.
  - AV is computed transposed: out[q, dh] psum tiles [128 queries, 65] with a
    ones-rider column in V accumulating the softmax denominator per query
    IN THE FREE DIM, so normalization is a per-partition reciprocal +
    scaled copy (no cross-partition broadcast, no DRAM round trip).
  - Output written as [S, 768] fp32 - already the final layout.
"""

import os
from contextlib import ExitStack

import numpy as np

NCLS, BS, NBLK, NPAIR, NH, DH = 1, 49, 32, 16, 12, 64
B, S, D = 8, 1569, 768
NTOK = S - NCLS  # 1568
N_CORES = 8
QK8 = int(__import__('os').environ.get('QK8', '1')) != 0
KEYS = 99            # 98 pair keys + 1 cls row
NSLOT = 16           # AV psum slots (2 blocks each, partitions 0-48 / 64-112)
CLSB = 1024 + 2 * 65  # cls-query corner col in the AV psum tile (bank 2)
AVW = CLSB + 65      # used width of the AV psum tile
SP8 = 1856           # per-dt stride of qT8/kT8 (zero strip beyond data)
KCOLS = NPAIR * KEYS  # 1584 pair-major key columns
QZ = 1576            # qT8 ktile1 landing col (zeros)
KZ = 1584            # kT8 ktile1 landing col (zeros)


# ----------------------------------------------------------------------------
# host-side layout
# ----------------------------------------------------------------------------

def _slot_target(qtok):
    """Map a query token to its AV psum target: (partition0, col_base).
    Slot s holds block 2s at partitions 0-48 and block 2s+1 at 64-112;
    the cls query (token 0) lives at partition 0 of the cls corner."""
    if qtok == 0:
        return 0, CLSB
    qb = (qtok - 1) // BS
    s = qb // 2
    cb = 65 * s if s < 7 else (512 + 65 * (s - 7) if s < 14 else 1024 + 65 * (s - 14))
    return 64 * (qb % 2) + (qtok - 1 - BS * qb), cb


def _build_layout(rand_idx):
    rand_idx = np.asarray(rand_idx)
    mult = np.zeros((NBLK, NBLK), np.int32)
    for m in range(NBLK):
        for o in (-1, 0, 1):
            mult[m, (m + o) % NBLK] += 1
        for r in rand_idx[m]:
            mult[m, int(r)] += 1

    # pack attending query columns per key-pair into banks of 512 (groups of
    # 1024). Block units (the cls col, or a 49-token block) never straddle a
    # bank boundary: the packing pads to the bank edge instead, so every AV
    # piece starts at an aligned psum partition (0 or 64). Pad columns carry
    # no scores; the bias matmul assigns them the -448 mask.
    segs = []
    gcol = 0
    for p in range(NPAIR):
        att = sorted(set(np.nonzero(mult[:, 2 * p])[0])
                     | set(np.nonzero(mult[:, 2 * p + 1])[0]))
        units = [(0, 1)] + [(1 + BS * m, BS) for m in att]
        cur = None
        prev_end = None
        for (uc, uw) in units:
            if 512 - (gcol % 512) < uw:
                gcol += 512 - (gcol % 512)  # pad to bank edge
                cur = None
            if cur is None or cur["bank"] != gcol // 512:
                cur = {"p": p, "runs": [], "bank": gcol // 512,
                       "off": gcol % 512}
                segs.append(cur)
                prev_end = None
            if prev_end == uc:
                rc0, rw0 = cur["runs"][-1]
                cur["runs"][-1] = (rc0, rw0 + uw)
            else:
                cur["runs"].append((uc, uw))
            prev_end = uc + uw
            gcol += uw
        cur = None  # next pair starts a new segment

    nbank = (gcol + 511) // 512
    ng = (nbank + 1) // 2
    for sg in segs:
        sg["g"] = sg["bank"] // 2
        sg["goff"] = (sg["bank"] % 2) * 512 + sg["off"]
    # group occupancy (incl. pad columns): all banks full except the last
    last_bank_fill = gcol - (nbank - 1) * 512
    gocc = []
    for g in range(ng):
        b0, b1 = 2 * g, 2 * g + 1
        occ = 0
        for b in (b0, b1):
            if b < nbank - 1:
                occ += 512
            elif b == nbank - 1:
                occ += last_bank_fill
        gocc.append(occ)

    # per-group score pieces (runs split to <=256), start flag per 512-region
    score_pieces = [[] for _ in range(ng)]
    bank_started = [False] * nbank
    for sg in segs:
        oc = 0
        for (rc, rw) in sg["runs"]:
            c, w = rc, rw
            while w > 0:
                take = min(w, 256)
                st = not bank_started[sg["bank"]]
                bank_started[sg["bank"]] = True
                score_pieces[sg["g"]].append(
                    (sg["p"], c, take, sg["goff"] + oc, st))
                oc += take
                c += take
                w -= take

    # bias pieces per group: cover each bank's occupancy in <=256 chunks,
    # split at the pad watermark (scores wrote [0, used); pads [used, bw) are
    # still pending-zero and must be covered by their own assign piece);
    # last chunk per bank carries stop
    bank_used = [0] * nbank
    for sg in segs:
        w = sum(rw for (_, rw) in sg["runs"])
        bank_used[sg["bank"]] = max(bank_used[sg["bank"]], sg["off"] + w)
    bias_pieces = [[] for _ in range(ng)]
    for g in range(ng):
        for half in range(2):
            b = 2 * g + half
            if b >= nbank:
                continue
            bw = 512 if b < nbank - 1 else last_bank_fill
            used = bank_used[b]
            c = 0
            while c < bw:
                lim = used if c < used else bw
                take = min(256, lim - c)
                bias_pieces[g].append(
                    (half * 512 + c, take, c + take >= bw))
                c += take

    # AV pieces per group: runs split at block units -> aligned partitions
    av_pieces = [[] for _ in range(ng)]
    for sg in segs:
        oc = 0
        for (rc, rw) in sg["runs"]:
            c, w = rc, rw
            while w > 0:
                take = 1 if c == 0 else min(w, BS - (c - 1) % BS)
                qp0, cb = _slot_target(c)
                av_pieces[sg["g"]].append(
                    ("vst", sg["p"], sg["goff"] + oc, take, qp0, cb))
                oc += take
                c += take
                w -= take

    # per-group column -> (qtok, pair) maps (qtok -1 = pad), cls designation
    lb_cols = []
    for g in range(ng):
        qtok = np.full(1024, -1, np.int64)
        pair = np.zeros(1024, np.int64)
        lb_cols.append((qtok, pair))
    cls_seen = np.zeros(S, bool)
    cls_des = np.zeros((ng, 1024), bool)
    for sg in segs:
        qtok, pair = lb_cols[sg["g"]]
        oc = sg["goff"]
        for (rc, rw) in sg["runs"]:
            qtok[oc:oc + rw] = np.arange(rc, rc + rw)
            pair[oc:oc + rw] = sg["p"]
            fresh = ~cls_seen[rc:rc + rw]
            cls_des[sg["g"], oc:oc + rw] = fresh
            cls_seen[rc:rc + rw] = True
            oc += rw

    # multiplicity extras: (qb, kb) with mult >= 2 -> (m-1) extra AV matmuls
    # against a half-masked V copy (vste slot per distinct (pair, half))
    vste_cases = []      # (pair, half)
    col_of = {}
    for sg in segs:
        oc = sg["goff"]
        for (rc, rw) in sg["runs"]:
            for i in range(rw):
                col_of[(sg["p"], rc + i)] = (sg["g"], oc + i)
            oc += rw
    for qb in range(NBLK):
        for kb in range(NBLK):
            m = int(mult[qb, kb])
            if m < 2:
                continue
            p, half = kb // 2, kb % 2
            if (p, half) not in vste_cases:
                vste_cases.append((p, half))
            e = vste_cases.index((p, half))
            t0 = 1 + BS * qb
            g, oc = col_of[(p, t0)]
            qp0, cb = _slot_target(t0)
            for _ in range(m - 1):
                av_pieces[g].append(("vste", e, oc, BS, qp0, cb))
    return {"segs": segs, "mult": mult, "ng": ng, "gocc": gocc,
            "nbank": nbank, "score_pieces": score_pieces,
            "bias_pieces": bias_pieces, "av_pieces": av_pieces,
            "lb_cols": lb_cols, "cls_des": cls_des,
            "vste_cases": vste_cases}


def _build_bias8(lay, rel_table, rel_pos_index, f8np):
    """lb8 [NH, ng, 50, 2048]: rows (p, i) -> key 49i+p (cls at (49,1)),
    values 16*bias, -240 where masked."""
    ng = lay["ng"]
    mult = lay["mult"]
    MASK = -240.0
    lb = np.full((NH, ng, 50, 2, 1024), MASK, np.float32)
    for sg in lay["segs"]:
        g = sg["g"]
        p = sg["p"]
        ktok = 1 + 98 * p + np.arange(98)          # [98]
        kblk = 2 * p + (np.arange(98) // BS)
        oc = sg["goff"]
        for (rc, rw) in sg["runs"]:
            qtok = np.arange(rc, rc + rw)
            qblk = np.maximum(qtok - 1, 0) // BS
            att = (mult[qblk][:, kblk] > 0) | (qtok == 0)[:, None]  # [rw, 98]
            idx = rel_pos_index[qtok[:, None], ktok[None, :]]       # [rw, 98]
            val = 16.0 * rel_table[idx]                             # [rw,98,NH]
            val = np.where(att[:, :, None], np.clip(val, -200, 200), MASK)
            v = val.transpose(2, 1, 0)                              # [NH,98,rw]
            lb[:, g, 0:49, 0, oc:oc + rw] = v[:, 0:49]
            lb[:, g, 0:49, 1, oc:oc + rw] = v[:, 49:98]
            # cls row: designated pair only
            des = lay["cls_des"][g, oc:oc + rw]
            cidx = rel_pos_index[qtok, 0]
            cval = np.clip(16.0 * rel_table[cidx], -200, 200)       # [rw, NH]
            cv = np.where(des[:, None], cval, MASK).T               # [NH, rw]
            lb[:, g, 49, 1, oc:oc + rw] = cv
            oc += rw
    return lb.reshape(NH, ng, 50, 2048).astype(f8np)


# ----------------------------------------------------------------------------
# walrus workaround: split the TileContext tail drain's sem waits
# ----------------------------------------------------------------------------

def _patch_tile_drain():
    import concourse.tile as tile
    from concourse.vector_clock import ScopedClock, VectorClock

    if getattr(tile.TileContext, "_beit_drain_patch", False):
        return

    def _drain_and_barrier(self, tick_clock, wait_clock):
        gc_vec = tick_clock.global_clock
        n = len(gc_vec)
        nonzero = [i for i in range(n) if gc_vec[i] > 0] or [0]
        for i in range(0, len(nonzero), 1):
            chunk = set(nonzero[i:i + 1])
            vec = VectorClock([gc_vec[j] if j in chunk else 0 for j in range(n)])
            drain_inst = self.nc.sync.drain()
            wait_clock.add_sem_waits(drain_inst.ins, ScopedClock({None: vec}))
        self.nc.all_engine_barrier()
        assert self.sems is not None
        popped = self.nc._tile_sem_poison_stack.pop()
        assert popped is self._sem_poison
        self.nc.clear_and_free_semaphores(list(self.sems.allocated().values()))
        self.nc.all_engine_barrier()

    tile.TileContext._drain_and_barrier = _drain_and_barrier
    tile.TileContext._beit_drain_patch = True


def _split_excess_waits(nc, mybir, limit=1):
    """This walrus build allows very few sem waits per instruction; move the
    excess onto EventSemaphore carrier instructions inserted just before."""
    ctr = [0]
    for f in nc.m.functions:
        for bb in f.blocks:
            il = bb.instructions
            out = []
            for inst in il:
                si = inst.sync_info
                if si is not None and si.on_wait and len(si.on_wait) > limit:
                    waits = list(si.on_wait)
                    over = waits[limit:]
                    for j in range(0, len(over), limit):
                        ctr[0] += 1
                        ev = mybir.InstEventSemaphore(
                            name=f"WSPLIT-{ctr[0]}", ins=[], outs=[],
                            engine=inst.engine,
                            sync_info=mybir.SyncInfo(on_wait=over[j:j + limit],
                                                     on_update=[]),
                        )
                        nc.register_instruction(ev, overwrite=True)
                        out.append(ev)
                    si.on_wait = waits[:limit]
                out.append(inst)
            il[:] = out
    return ctr[0]


# ----------------------------------------------------------------------------
# device kernel emission
# ----------------------------------------------------------------------------

def _emit(nc, tile, mybir, lay):
    import concourse.bass as bass

    bf = mybir.dt.bfloat16
    f8 = mybir.dt.float8e4
    f32 = mybir.dt.float32
    DR = mybir.MatmulPerfMode.DoubleRow
    Exp = mybir.ActivationFunctionType.Exp
    ng = lay["ng"]
    NE = max(1, len(lay["vste_cases"]))

    f16 = mybir.dt.float16
    hs16_d = nc.dram_tensor("hs16", [D, S], f16, kind="ExternalInput")
    hsk16_d = nc.dram_tensor("hsk16", [D, KCOLS], f16, kind="ExternalInput")
    wq16_d = nc.dram_tensor("Wq16", [D, D], f16, kind="ExternalInput")
    wk16_d = nc.dram_tensor("Wk16", [D, D], f16, kind="ExternalInput")
    wv16_d = nc.dram_tensor("Wv16", [D, D], f16, kind="ExternalInput")
    bq8_d = nc.dram_tensor("bq8", [1, 1024], f8, kind="ExternalInput")
    bv8_d = nc.dram_tensor("bv8", [1, 1024], f8, kind="ExternalInput")
    i8_d = nc.dram_tensor("I8", [50, 2 * KEYS], f8, kind="ExternalInput")
    cz16_d = nc.dram_tensor("cz16", [1, 2048], bf, kind="ExternalInput")
    lb8_d = nc.dram_tensor("lb8", [NH, ng, 50, 2048], f8, kind="ExternalInput")
    out_d = nc.dram_tensor("out_s", [S, D], f32, kind="ExternalOutput")

    def ap3(sl, s1, n1, s2, n2):
        return bass.AP(tensor=sl.tensor, offset=sl.offset,
                       ap=[list(sl.ap[0]), [s1, n1], [s2, n2]])

    def slot_col(s):
        return 65 * s if s < 7 else (512 + 65 * (s - 7) if s < 14
                                     else 1024 + 65 * (s - 14))

    av_banks = [(0, 455), (512, 455), (1024, AVW - 1024)]

    with tile.TileContext(nc) as tc, ExitStack() as ctx:
        consts = ctx.enter_context(tc.tile_pool(name="consts", bufs=1))
        persist = ctx.enter_context(tc.tile_pool(name="persist", bufs=1))

        c64 = consts.tile([1, 1024], f8, tag="c64", name="c64")
        nc.vector.memset(c64[:, :], 1.0 / 64.0)
        o8c = consts.tile([1, 1024], f8, tag="o8c", name="o8c")
        nc.vector.memset(o8c[:, 0:512], 1.0 / 64.0)
        nc.vector.memset(o8c[:, 512:1024], 0.0)
        bq8 = consts.tile([1, 1024], f8, tag="bq8", name="bq8")
        nc.gpsimd.dma_start(out=bq8[:, :], in_=bq8_d[:, :])
        bv8 = consts.tile([1, 1024], f8, tag="bv8", name="bv8")
        nc.gpsimd.dma_start(out=bv8[:, :], in_=bv8_d[:, :])
        i8sb = consts.tile([50, 2 * KEYS], f8, tag="i8", name="i8")
        nc.gpsimd.dma_start(out=i8sb[:, :], in_=i8_d[:, :])

        qkdt = f8 if QK8 else bf
        qT8 = persist.tile([128, 6 * SP8], qkdt, tag="qT8", name="qT8")
        kT8 = persist.tile([128, 6 * SP8], qkdt, tag="kT8", name="kT8")
        for t in range(6):
            nc.vector.memset(qT8[:, t * SP8 + S:(t + 1) * SP8], 0.0)
            nc.vector.memset(kT8[:, t * SP8 + KCOLS:(t + 1) * SP8], 0.0)
        vst = persist.tile([KEYS, NPAIR * NH * 65], bf, tag="vst", name="vst")
        vst4 = vst[:, :].rearrange("a (p h e) -> a p h e", p=NPAIR, h=NH)
        nc.vector.memset(vst4[:, :, :, 64:65], 1.0)
        vste = persist.tile([KEYS, NE * NH * 65], bf, tag="vste", name="vste")
        nc.gpsimd.memset(vste[:, :], 0.0)
        vste4 = vste[:, :].rearrange("a (e h c) -> a e h c", e=NE, h=NH)
        def bcast49(dram_sl, inner):
            # DRAM source broadcast across 49 partitions
            return bass.AP(tensor=dram_sl.tensor, offset=dram_sl.offset,
                           ap=[[0, 49]] + inner)
        outS = persist.tile([128, (NSLOT + 1) * D], f32, tag="outS", name="outS")

        # ---------------- phase A: projections ----------------
        with tc.tile_pool(name="phA", bufs=1) as phA, \
             tc.tile_pool(name="pp", bufs=3, space="PSUM") as pp, \
             tc.tile_pool(name="ppv", bufs=2, space="PSUM") as ppv:
            hs16, hsk16 = [], []
            w_sb = {"q": [], "k": [], "v": []}
            for t in range(6):
                ht = phA.tile([128, S], f16, tag=f"hs16_{t}", name=f"hs16_{t}")
                nc.sync.dma_start(out=ht[:, :], in_=hs16_d[t * 128:(t + 1) * 128, :])
                hs16.append(ht)
                wt = phA.tile([128, D], f16, tag=f"wq16_{t}", name=f"wq16_{t}")
                nc.sync.dma_start(out=wt[:, :], in_=wq16_d[t * 128:(t + 1) * 128, :])
                w_sb["q"].append(wt)
            for t in range(6):
                ht = phA.tile([128, KCOLS], f16, tag=f"hsk16_{t}", name=f"hsk16_{t}")
                nc.gpsimd.dma_start(out=ht[:, :], in_=hsk16_d[t * 128:(t + 1) * 128, :])
                hsk16.append(ht)
            for nm, dram in (("k", wk16_d), ("v", wv16_d)):
                for t in range(6):
                    wt = phA.tile([128, D], f16, tag=f"w{nm}16_{t}", name=f"w{nm}16_{t}")
                    nc.gpsimd.dma_start(out=wt[:, :],
                                        in_=dram[t * 128:(t + 1) * 128, :])
                    w_sb[nm].append(wt)

            # qT8 / kT8 projections (fp16 matmuls, fp8 store): psum [128, chunk]
            qchunks = [(0, 512), (512, 512), (1024, 512), (1536, S - 1536)]
            kchunks = [(0, 512), (512, 512), (1024, 512), (1536, KCOLS - 1536)]
            for name, hsrc, chunks, dst, scale in (
                    ("q", hs16, qchunks, qT8, 0.5),
                    ("k", hsk16, kchunks, kT8, 0.25)):
                for dt in range(6):
                    for (c0, cw) in chunks:
                        ps = pp.tile([128, 512], f32, tag="pq", name="pq")
                        for kt in range(6):
                            nc.tensor.matmul(
                                ps[:, 0:cw],
                                lhsT=w_sb[name][kt][:, dt * 128:(dt + 1) * 128],
                                rhs=hsrc[kt][:, c0:c0 + cw],
                                start=(kt == 0),
                                stop=(kt == 5 and name == "k"))
                        if name == "q":
                            s0 = 0
                            while s0 < cw:
                                sw = min(256, cw - s0)
                                lhsT = ap3(bq8[0:1, dt * 128:dt * 128 + 1],
                                           D - dt * 128, 2, 1, 128)
                                rhs = ap3(c64[0:1, 0:1], 512, 2, 1, sw)
                                nc.tensor.matmul(ps[:, s0:s0 + sw], lhsT=lhsT,
                                                 rhs=rhs, start=False,
                                                 stop=(s0 + sw >= cw),
                                                 perf_mode=DR)
                                s0 += sw
                        nc.vector.tensor_scalar_mul(
                            dst[:, dt * SP8 + c0:dt * SP8 + c0 + cw],
                            ps[:, 0:cw], scale)

            # V projection per pair (pair-major hs incl. the cls-dup column,
            # so row 98 of each pair slice is v_cls): psum [99 tokens, 768]
            ecase = {pc: e for e, pc in enumerate(lay["vste_cases"])}
            for p in range(NPAIR):
                c0 = KEYS * p
                ps = ppv.tile([128, D], f32, tag="pv", name="pv")
                for (h0, hw_) in ((0, 512), (512, 256)):
                    for kt in range(6):
                        nc.tensor.matmul(
                            ps[0:KEYS, h0:h0 + hw_],
                            lhsT=hsk16[kt][:, c0:c0 + KEYS],
                            rhs=w_sb["v"][kt][:, h0:h0 + hw_],
                            start=(kt == 0), stop=False)
                    s0 = h0
                    while s0 < h0 + hw_:
                        sw = min(256, h0 + hw_ - s0)
                        lhsT = ap3(o8c[0:1, 0:1], 512, 2, 1, KEYS)
                        rhs = ap3(bv8[0:1, s0:s0 + 1], D - s0, 2, 1, sw)
                        nc.tensor.matmul(ps[0:KEYS, s0:s0 + sw], lhsT=lhsT,
                                         rhs=rhs, start=False,
                                         stop=(s0 + sw >= h0 + hw_),
                                         perf_mode=DR)
                        s0 += sw
                src = ps[0:KEYS, :].rearrange("a (h e) -> a h e", h=NH)
                nc.gpsimd.tensor_copy(vst4[0:KEYS, p, :, 0:64], src)
                for half in range(2):
                    if (p, half) not in ecase:
                        continue
                    e = ecase[(p, half)]
                    nc.gpsimd.tensor_copy(vste4[0:98, e, :, 0:64],
                                          ps[0:98, :].rearrange(
                                              "a (h e) -> a h e", h=NH))
                    if half == 0:
                        # zero the inactive upper half, rider=1 on lower
                        nc.sync.dma_start(out=vste4[49:98, e, :, 0:65],
                                          in_=bcast49(cz16_d[0:1, 0:1],
                                                      [[65, NH], [1, 65]]))
                        nc.gpsimd.memset(vste4[0:49, e, :, 64:65], 1.0)
                    else:
                        # zero the inactive lower half, rider=1 on upper
                        nc.gpsimd.memset(vste4[0:49, e, :, 0:65], 0.0)
                        nc.sync.dma_start(
                            out=vste4[49:98, e, :, 64:65],
                            in_=bcast49(cz16_d[0:1, 1024:1025],
                                        [[1, NH], [1, 1]]))

        # ---------------- phase B: block-sparse attention per head ----------
        with tc.tile_pool(name="scps", bufs=2, space="PSUM") as scps, \
             tc.tile_pool(name="avps", bufs=1, space="PSUM") as avps, \
             tc.tile_pool(name="ab", bufs=3) as ab, \
             tc.tile_pool(name="lbp", bufs=3) as lbp, \
             tc.tile_pool(name="nrm", bufs=2) as nrm:

            def emit_openers(avt):
                # init every used AV psum byte to a tiny value ((1/64)^2 * 2)
                for (b0, bw) in av_banks:
                    first = True
                    c = 0
                    while c < bw:
                        take = min(256, bw - c)
                        nc.tensor.matmul(
                            avt[:, b0 + c:b0 + c + take],
                            lhsT=ap3(c64[0:1, 0:1], 512, 2, 1, 128),
                            rhs=ap3(c64[0:1, 0:1], 512, 2, 1, take),
                            start=first, stop=False, perf_mode=DR)
                        first = False
                        c += take

            def emit_av(h, g, aT, avt):
                for (kind, pe, oc, w, qp0, cb) in lay["av_pieces"][g]:
                    if kind == "vst":
                        rhs = vst4[0:KEYS, pe, h, 0:65]
                    else:
                        rhs = vste4[0:KEYS, pe, h, 0:65]
                    nc.tensor.matmul(
                        avt[qp0:qp0 + w, cb:cb + 65],
                        lhsT=aT[0:KEYS, oc:oc + w], rhs=rhs,
                        start=False, stop=False)

            def emit_head_tail(h, avt):
                # 1-col closers: end each bank's group on all 128 partitions
                for (b0, bw) in av_banks:
                    nc.tensor.matmul(avt[:, b0:b0 + 1],
                                     lhsT=ap3(c64[0:1, 0:1], 512, 2, 1, 128),
                                     rhs=ap3(c64[0:1, 0:1], 512, 2, 1, 1),
                                     start=False, stop=True, perf_mode=DR)
                rcol = nrm.tile([128, 17], f32, tag="rcol", name="rcol")
                for (i0, i1, base, n) in ((0, 7, 64, 7),
                                          (7, 14, 512 + 64, 7),
                                          (14, 17, 1024 + 64, 3)):
                    d0 = avt[:, base:base + 1]
                    nc.vector.reciprocal(
                        rcol[:, i0:i1],
                        bass.AP(tensor=d0.tensor, offset=d0.offset,
                                ap=[list(d0.ap[0]), [65, n]]))
                for s in range(NSLOT):
                    eng = nc.vector if s % 2 == 0 else nc.gpsimd
                    eng.tensor_scalar_mul(
                        outS[:, s * D + h * DH:s * D + h * DH + DH],
                        avt[:, slot_col(s):slot_col(s) + 64],
                        rcol[:, s:s + 1])
                nc.vector.tensor_scalar_mul(
                    outS[0:1, NSLOT * D + h * DH:NSLOT * D + h * DH + DH],
                    avt[0:1, CLSB:CLSB + 64], rcol[0:1, 16:17])

            pending = None
            avt_by_h = {}
            for h in range(NH):
                dt, r0 = h // 2, 64 * (h % 2)
                avt = avps.tile([128, 1536], f32, tag="avt", name="avt")
                avt_by_h[h] = avt
                for g in range(ng):
                    gw = lay["gocc"][g]
                    sc = scps.tile([128, 1024], f32, tag="sc", name="sc")
                    lb = lbp.tile([50, 2048], f8, tag="lb", name="lb")
                    nc.sync.dma_start(out=lb[:, :], in_=lb8_d[h, g])
                    for (p, rc, rw, oc, st) in lay["score_pieces"][g]:
                        if QK8:
                            lhsT = ap3(kT8[r0:r0 + 64, dt * SP8 + 99 * p:dt * SP8 + 99 * p + 1],
                                       KZ - 99 * p, 2, 1, KEYS)
                            rhs = ap3(qT8[r0:r0 + 64, dt * SP8 + rc:dt * SP8 + rc + 1],
                                      QZ - rc, 2, 1, rw)
                            nc.tensor.matmul(sc[0:KEYS, oc:oc + rw], lhsT=lhsT,
                                             rhs=rhs, start=st, stop=False,
                                             perf_mode=DR)
                        else:
                            nc.tensor.matmul(
                                sc[0:KEYS, oc:oc + rw],
                                lhsT=kT8[r0:r0 + 64, dt * SP8 + 99 * p:dt * SP8 + 99 * p + KEYS],
                                rhs=qT8[r0:r0 + 64, dt * SP8 + rc:dt * SP8 + rc + rw],
                                start=st, stop=False)
                    for (bc0, bw, sp) in lay["bias_pieces"][g]:
                        lhsT = ap3(i8sb[0:50, 0:1], KEYS, 2, 1, KEYS)
                        rhs = ap3(lb[0:50, bc0:bc0 + 1], 1024, 2, 1, bw)
                        nc.tensor.matmul(sc[0:KEYS, bc0:bc0 + bw], lhsT=lhsT,
                                         rhs=rhs, start=False, stop=sp,
                                         perf_mode=DR)
                    aT = ab.tile([KEYS, 1024], bf, tag="aT", name="aT")
                    nc.scalar.activation(aT[:, 0:gw], sc[0:KEYS, 0:gw], Exp)
                    if pending is not None:
                        ph, pg, paT = pending
                        if pg == 0:
                            emit_openers(avt_by_h[ph])
                        emit_av(ph, pg, paT, avt_by_h[ph])
                        if pg == ng - 1:
                            emit_head_tail(ph, avt_by_h.pop(ph))
                    pending = (h, g, aT)
            ph, pg, paT = pending
            if pg == 0:
                emit_openers(avt_by_h[ph])
            emit_av(ph, pg, paT, avt_by_h[ph])
            emit_head_tail(ph, avt_by_h.pop(ph))

            # output DMA per slot half (+ cls token row)
            for s in range(NSLOT):
                t0 = 1 + 98 * s
                nc.gpsimd.dma_start(out=out_d[t0:t0 + 49, :],
                                    in_=outS[0:49, s * D:(s + 1) * D])
                nc.gpsimd.dma_start(out=out_d[t0 + 49:t0 + 98, :],
                                    in_=outS[64:113, s * D:(s + 1) * D])
            nc.gpsimd.dma_start(out=out_d[0:1, :],
                                in_=outS[0:1, NSLOT * D:(NSLOT + 1) * D])

    _split_excess_waits(nc, mybir, limit=1)
    return nc


# ----------------------------------------------------------------------------
# host-side input prep
# ----------------------------------------------------------------------------

def _prepare(hidden_states, Wq, bq, Wk, Wv, bv, rel_table, rel_pos_index, rand_idx):
    import ml_dtypes

    import concourse.bass as bass
    import concourse.tile as tile
    from concourse import mybir

    _patch_tile_drain()
    f8np = ml_dtypes.float8_e4m3

    hidden_states = np.asarray(hidden_states, np.float32)
    Wq = np.asarray(Wq, np.float32)
    Wk = np.asarray(Wk, np.float32)
    Wv = np.asarray(Wv, np.float32)
    bq = np.asarray(bq, np.float32)
    bv = np.asarray(bv, np.float32)
    rel_table = np.asarray(rel_table, np.float32)
    rel_pos_index = np.asarray(rel_pos_index)
    rand_idx = np.asarray(rand_idx)

    lay = _build_layout(rand_idx)
    lb8 = _build_bias8(lay, rel_table, rel_pos_index, f8np)

    i8 = np.zeros((50, 2, KEYS), np.float32)
    for p in range(49):
        i8[p, 0, p] = 1.0 / 16.0
        i8[p, 1, 49 + p] = 1.0 / 16.0
    i8[49, 1, 98] = 1.0 / 16.0

    f16np = np.float16
    shared = {
        "Wq16": Wq.astype(f16np), "Wk16": Wk.astype(f16np),
        "Wv16": Wv.astype(f16np),
        "bq8": np.concatenate([bq * 64.0, np.zeros(256, np.float32)]
                              ).reshape(1, 1024).astype(f8np),
        "bv8": np.concatenate([bv * 64.0, np.zeros(256, np.float32)]
                              ).reshape(1, 1024).astype(f8np),
        "I8": i8.reshape(50, 2 * KEYS).astype(f8np),
        "lb8": lb8,
        "cz16": np.concatenate([np.zeros(1024, np.float32),
                                np.ones(1024, np.float32)]
                               ).reshape(1, 2048).astype(ml_dtypes.bfloat16),
    }

    # pair-major token order for the k/v projections (cls duplicated per pair)
    korder = np.empty(KCOLS, np.int64)
    for p in range(NPAIR):
        korder[99 * p:99 * p + 98] = 1 + 98 * p + np.arange(98)
        korder[99 * p + 98] = 0

    in_maps = []
    for b in range(B):
        hsT = hidden_states[b].T  # [768, S]
        m = dict(shared)
        m["hs16"] = np.ascontiguousarray(hsT).astype(f16np)
        m["hsk16"] = np.ascontiguousarray(hsT[:, korder]).astype(f16np)
        in_maps.append(m)

    nc = bass.Bass()
    _emit(nc, tile, mybir, lay)
    return nc, in_maps


# ----------------------------------------------------------------------------
# optional PJRT repeat-bench (unused by default; kept from v1)
# ----------------------------------------------------------------------------

def _bench_pjrt(nc, in_maps, n_cores, iters=20, warmup=3):
    import time

    import jax
    from jax.sharding import Mesh, PartitionSpec
    from jax.experimental.shard_map import shard_map

    from concourse import mybir
    from concourse.bass2jax import (_bass_exec_p, install_neuronx_cc_hook,
                                    partition_id_tensor)

    install_neuronx_cc_hook()
    partition_name = nc.partition_id_tensor.name if nc.partition_id_tensor else None
    in_names, out_names, out_avals, zero_outs = [], [], [], []
    for alloc in nc.m.functions[0].allocations:
        if not isinstance(alloc, mybir.MemoryLocationSet):
            continue
        name = alloc.memorylocations[0].name
        if alloc.kind == "ExternalInput":
            if name != partition_name:
                in_names.append(name)
        elif alloc.kind == "ExternalOutput":
            shape = tuple(alloc.tensor_shape)
            dtype = mybir.dt.np(alloc.dtype)
            out_names.append(name)
            out_avals.append(jax.core.ShapedArray(shape, dtype))
            zero_outs.append(np.zeros(shape, dtype))
    n_params = len(in_names)
    all_in_names = in_names + out_names + ([partition_name] if partition_name else [])

    def _body(*args):
        operands = list(args)
        if partition_name is not None:
            operands.append(partition_id_tensor())
        return tuple(_bass_exec_p.bind(
            *operands,
            out_avals=tuple(out_avals),
            in_names=tuple(all_in_names),
            out_names=tuple(out_names),
            lowering_input_output_aliases=(),
            sim_require_finite=True,
            sim_require_nnan=True,
            nc=nc,
        ))

    devices = jax.devices()[:n_cores]
    mesh = Mesh(np.asarray(devices), ("core",))
    n_outs = len(out_names)
    sharded = jax.jit(
        shard_map(_body, mesh=mesh,
                  in_specs=(PartitionSpec("core"),) * (n_params + n_outs),
                  out_specs=(PartitionSpec("core"),) * n_outs,
                  check_rep=False),
        keep_unused=True,
    )
    per_core = [[np.asarray(m[name]) for name in in_names] for m in in_maps]
    concat_in = [np.concatenate([per_core[c][i] for c in range(n_cores)], axis=0)
                 for i in range(n_params)]
    concat_zeros = [np.zeros((n_cores * z.shape[0], *z.shape[1:]), z.dtype)
                    for z in zero_outs]
    dev_in = [jax.device_put(a) for a in concat_in + concat_zeros]
    out = sharded(*dev_in)
    jax.block_until_ready(out)
    for _ in range(warmup):
        out = sharded(*dev_in)
    jax.block_until_ready(out)
    t0 = time.perf_counter()
    for _ in range(iters):
        out = sharded(*dev_in)
    jax.block_until_ready(out)
    dt = (time.perf_counter() - t0) / iters
    results = [
        {name: np.asarray(out[i]).reshape(n_cores, *out_avals[i].shape)[c]
         for i, name in enumerate(out_names)}
        for c in range(n_cores)
    ]
    return int(dt * 1e9), results


# ----------------------------------------------------------------------------
# public entry point
# ----------------------------------------------------------------------------

def kernel(hidden_states, Wq, bq, Wk, Wv, bv, rel_table, rel_pos_index, rand_idx):
    from concourse.bass_utils import run_bass_kernel_spmd

    nc, in_maps = _prepare(hidden_states, Wq, bq, Wk, Wv, bv,
                           rel_table, rel_pos_index, rand_idx)

    kernel.last_nc = nc
    kernel.last_in_maps = in_maps
    bench_iters = int(os.environ.get("BEIT_BENCH", "0"))
    if bench_iters > 0:
        per_iter_ns, results = _bench_pjrt(nc, in_maps, N_CORES, iters=bench_iters)
        kernel.last_exec_time_ns = per_iter_ns
    else:
        res = run_bass_kernel_spmd(nc, in_maps, core_ids=list(range(N_CORES)))
        results = res.results

    out = np.empty((B, S, D), np.float32)
    for b in range(B):
        out[b] = results[b]["out_s"]
    return out


# revision 30
# speedup vs baseline: 1.0192x; 1.0192x over previous
"""BeitSelfAttention block-sparse attention kernel for 8 Trainium2 NeuronCores.

Strategy (data-parallel over batch, B=8 -> one batch element per core):
  - fp8e4 DoubleRow matmuls for QKV projections and block-sparse scores
    (two 128-row k-tiles per pass at 0.5 cycles/row).
  - The relative-position bias AND the block-sparsity mask are host-packed
    into one fp8 table (16x scale) and added into the score PSUM by an
    identity DoubleRow matmul; gather multiplicity (rand/local block
    collisions) is realized by a few extra AV matmuls against half-masked V
    copies, so no per-element multiply is needed on DVE at all.
  - The cls KEY rides as a 99th score row per key-pair tile (designated to
    one pair per query via the bias mask); the cls QUERY is packed column 0.
  - AV is computed transposed: out[q, dh] psum tiles [128 queries, 65] with a
    ones-rider column in V accumulating the softmax denominator per query
    IN THE FREE DIM, so normalization is a per-partition reciprocal +
    scaled copy (no cross-partition broadcast, no DRAM round trip).
  - Output written as [S, 768] fp32 - already the final layout.
"""

import os
from contextlib import ExitStack

import numpy as np

NCLS, BS, NBLK, NPAIR, NH, DH = 1, 49, 32, 16, 12, 64
B, S, D = 8, 1569, 768
NTOK = S - NCLS  # 1568
N_CORES = 8
QK8 = int(__import__('os').environ.get('QK8', '1')) != 0
KEYS = 99            # 98 pair keys + 1 cls row
NSLOT = 16           # AV psum slots (2 blocks each, partitions 0-48 / 64-112)
CLSB = 1024 + 2 * 65  # cls-query corner col in the AV psum tile (bank 2)
AVW = CLSB + 65      # used width of the AV psum tile
SP8 = 1856           # per-dt stride of qT8/kT8 (zero strip beyond data)
KCOLS = NPAIR * KEYS  # 1584 pair-major key columns
QZ = 1576            # qT8 ktile1 landing col (zeros)
KZ = 1584            # kT8 ktile1 landing col (zeros)


# ----------------------------------------------------------------------------
# host-side layout
# ----------------------------------------------------------------------------

def _slot_target(qtok):
    """Map a query token to its AV psum target: (partition0, col_base).
    Slot s holds block 2s at partitions 0-48 and block 2s+1 at 64-112;
    the cls query (token 0) lives at partition 0 of the cls corner."""
    if qtok == 0:
        return 0, CLSB
    qb = (qtok - 1) // BS
    s = qb // 2
    cb = 65 * s if s < 7 else (512 + 65 * (s - 7) if s < 14 else 1024 + 65 * (s - 14))
    return 64 * (qb % 2) + (qtok - 1 - BS * qb), cb


def _build_layout(rand_idx):
    rand_idx = np.asarray(rand_idx)
    mult = np.zeros((NBLK, NBLK), np.int32)
    for m in range(NBLK):
        for o in (-1, 0, 1):
            mult[m, (m + o) % NBLK] += 1
        for r in rand_idx[m]:
            mult[m, int(r)] += 1

    # pack attending query columns per key-pair into banks of 512 (groups of
    # 1024). Block units (the cls col, or a 49-token block) never straddle a
    # bank boundary: the packing pads to the bank edge instead, so every AV
    # piece starts at an aligned psum partition (0 or 64). Pad columns carry
    # no scores; the bias matmul assigns them the -448 mask.
    segs = []
    gcol = 0
    for p in range(NPAIR):
        att = sorted(set(np.nonzero(mult[:, 2 * p])[0])
                     | set(np.nonzero(mult[:, 2 * p + 1])[0]))
        units = [(0, 1)] + [(1 + BS * m, BS) for m in att]
        cur = None
        prev_end = None
        for (uc, uw) in units:
            if 512 - (gcol % 512) < uw:
                gcol += 512 - (gcol % 512)  # pad to bank edge
                cur = None
            if cur is None or cur["bank"] != gcol // 512:
                cur = {"p": p, "runs": [], "bank": gcol // 512,
                       "off": gcol % 512}
                segs.append(cur)
                prev_end = None
            if prev_end == uc:
                rc0, rw0 = cur["runs"][-1]
                cur["runs"][-1] = (rc0, rw0 + uw)
            else:
                cur["runs"].append((uc, uw))
            prev_end = uc + uw
            gcol += uw
        cur = None  # next pair starts a new segment

    nbank = (gcol + 511) // 512
    ng = (nbank + 1) // 2
    for sg in segs:
        sg["g"] = sg["bank"] // 2
        sg["goff"] = (sg["bank"] % 2) * 512 + sg["off"]
    # group occupancy (incl. pad columns): all banks full except the last
    last_bank_fill = gcol - (nbank - 1) * 512
    gocc = []
    for g in range(ng):
        b0, b1 = 2 * g, 2 * g + 1
        occ = 0
        for b in (b0, b1):
            if b < nbank - 1:
                occ += 512
            elif b == nbank - 1:
                occ += last_bank_fill
        gocc.append(occ)

    # per-group score pieces (runs split to <=256), start flag per 512-region
    score_pieces = [[] for _ in range(ng)]
    bank_started = [False] * nbank
    for sg in segs:
        oc = 0
        for (rc, rw) in sg["runs"]:
            c, w = rc, rw
            while w > 0:
                take = min(w, 256)
                st = not bank_started[sg["bank"]]
                bank_started[sg["bank"]] = True
                score_pieces[sg["g"]].append(
                    (sg["p"], c, take, sg["goff"] + oc, st))
                oc += take
                c += take
                w -= take

    # bias pieces per group: cover each bank's occupancy in <=256 chunks,
    # split at the pad watermark (scores wrote [0, used); pads [used, bw) are
    # still pending-zero and must be covered by their own assign piece);
    # last chunk per bank carries stop
    bank_used = [0] * nbank
    for sg in segs:
        w = sum(rw for (_, rw) in sg["runs"])
        bank_used[sg["bank"]] = max(bank_used[sg["bank"]], sg["off"] + w)
    bias_pieces = [[] for _ in range(ng)]
    for g in range(ng):
        for half in range(2):
            b = 2 * g + half
            if b >= nbank:
                continue
            bw = 512 if b < nbank - 1 else last_bank_fill
            used = bank_used[b]
            c = 0
            while c < bw:
                lim = used if c < used else bw
                take = min(256, lim - c)
                bias_pieces[g].append(
                    (half * 512 + c, take, c + take >= bw))
                c += take

    # AV pieces per group: runs split at block units -> aligned partitions
    av_pieces = [[] for _ in range(ng)]
    for sg in segs:
        oc = 0
        for (rc, rw) in sg["runs"]:
            c, w = rc, rw
            while w > 0:
                take = 1 if c == 0 else min(w, BS - (c - 1) % BS)
                qp0, cb = _slot_target(c)
                av_pieces[sg["g"]].append(
                    ("vst", sg["p"], sg["goff"] + oc, take, qp0, cb))
                oc += take
                c += take
                w -= take

    # per-group column -> (qtok, pair) maps (qtok -1 = pad), cls designation
    lb_cols = []
    for g in range(ng):
        qtok = np.full(1024, -1, np.int64)
        pair = np.zeros(1024, np.int64)
        lb_cols.append((qtok, pair))
    cls_seen = np.zeros(S, bool)
    cls_des = np.zeros((ng, 1024), bool)
    for sg in segs:
        qtok, pair = lb_cols[sg["g"]]
        oc = sg["goff"]
        for (rc, rw) in sg["runs"]:
            qtok[oc:oc + rw] = np.arange(rc, rc + rw)
            pair[oc:oc + rw] = sg["p"]
            fresh = ~cls_seen[rc:rc + rw]
            cls_des[sg["g"], oc:oc + rw] = fresh
            cls_seen[rc:rc + rw] = True
            oc += rw

    # multiplicity extras: (qb, kb) with mult >= 2 -> (m-1) extra AV matmuls
    # against a half-masked V copy (vste slot per distinct (pair, half))
    vste_cases = []      # (pair, half)
    col_of = {}
    for sg in segs:
        oc = sg["goff"]
        for (rc, rw) in sg["runs"]:
            for i in range(rw):
                col_of[(sg["p"], rc + i)] = (sg["g"], oc + i)
            oc += rw
    for qb in range(NBLK):
        for kb in range(NBLK):
            m = int(mult[qb, kb])
            if m < 2:
                continue
            p, half = kb // 2, kb % 2
            if (p, half) not in vste_cases:
                vste_cases.append((p, half))
            e = vste_cases.index((p, half))
            t0 = 1 + BS * qb
            g, oc = col_of[(p, t0)]
            qp0, cb = _slot_target(t0)
            for _ in range(m - 1):
                av_pieces[g].append(("vste", e, oc, BS, qp0, cb))
    return {"segs": segs, "mult": mult, "ng": ng, "gocc": gocc,
            "nbank": nbank, "score_pieces": score_pieces,
            "bias_pieces": bias_pieces, "av_pieces": av_pieces,
            "lb_cols": lb_cols, "cls_des": cls_des,
            "vste_cases": vste_cases}


def _build_bias8(lay, rel_table, rel_pos_index, f8np):
    """lb8 [NH, ng, 50, 2048]: rows (p, i) -> key 49i+p (cls at (49,1)),
    values 16*bias, -240 where masked."""
    ng = lay["ng"]
    mult = lay["mult"]
    MASK = -240.0
    lb = np.full((NH, ng, 50, 2, 1024), MASK, np.float32)
    for sg in lay["segs"]:
        g = sg["g"]
        p = sg["p"]
        ktok = 1 + 98 * p + np.arange(98)          # [98]
        kblk = 2 * p + (np.arange(98) // BS)
        oc = sg["goff"]
        for (rc, rw) in sg["runs"]:
            qtok = np.arange(rc, rc + rw)
            qblk = np.maximum(qtok - 1, 0) // BS
            att = (mult[qblk][:, kblk] > 0) | (qtok == 0)[:, None]  # [rw, 98]
            idx = rel_pos_index[qtok[:, None], ktok[None, :]]       # [rw, 98]
            val = 16.0 * rel_table[idx]                             # [rw,98,NH]
            val = np.where(att[:, :, None], np.clip(val, -200, 200), MASK)
            v = val.transpose(2, 1, 0)                              # [NH,98,rw]
            lb[:, g, 0:49, 0, oc:oc + rw] = v[:, 0:49]
            lb[:, g, 0:49, 1, oc:oc + rw] = v[:, 49:98]
            # cls row: designated pair only
            des = lay["cls_des"][g, oc:oc + rw]
            cidx = rel_pos_index[qtok, 0]
            cval = np.clip(16.0 * rel_table[cidx], -200, 200)       # [rw, NH]
            cv = np.where(des[:, None], cval, MASK).T               # [NH, rw]
            lb[:, g, 49, 1, oc:oc + rw] = cv
            oc += rw
    return lb.reshape(NH, ng, 50, 2048).astype(f8np)


# ----------------------------------------------------------------------------
# walrus workaround: split the TileContext tail drain's sem waits
# ----------------------------------------------------------------------------

def _patch_tile_drain():
    import concourse.tile as tile
    from concourse.vector_clock import ScopedClock, VectorClock

    if getattr(tile.TileContext, "_beit_drain_patch", False):
        return

    def _drain_and_barrier(self, tick_clock, wait_clock):
        gc_vec = tick_clock.global_clock
        n = len(gc_vec)
        nonzero = [i for i in range(n) if gc_vec[i] > 0] or [0]
        for i in range(0, len(nonzero), 1):
            chunk = set(nonzero[i:i + 1])
            vec = VectorClock([gc_vec[j] if j in chunk else 0 for j in range(n)])
            drain_inst = self.nc.sync.drain()
            wait_clock.add_sem_waits(drain_inst.ins, ScopedClock({None: vec}))
        self.nc.all_engine_barrier()
        assert self.sems is not None
        popped = self.nc._tile_sem_poison_stack.pop()
        assert popped is self._sem_poison
        self.nc.clear_and_free_semaphores(list(self.sems.allocated().values()))
        self.nc.all_engine_barrier()

    tile.TileContext._drain_and_barrier = _drain_and_barrier
    tile.TileContext._beit_drain_patch = True


def _split_excess_waits(nc, mybir, limit=1):
    """This walrus build allows very few sem waits per instruction; move the
    excess onto EventSemaphore carrier instructions inserted just before."""
    ctr = [0]
    for f in nc.m.functions:
        for bb in f.blocks:
            il = bb.instructions
            out = []
            for inst in il:
                si = inst.sync_info
                if si is not None and si.on_wait and len(si.on_wait) > limit:
                    waits = list(si.on_wait)
                    over = waits[limit:]
                    for j in range(0, len(over), limit):
                        ctr[0] += 1
                        ev = mybir.InstEventSemaphore(
                            name=f"WSPLIT-{ctr[0]}", ins=[], outs=[],
                            engine=inst.engine,
                            sync_info=mybir.SyncInfo(on_wait=over[j:j + limit],
                                                     on_update=[]),
                        )
                        nc.register_instruction(ev, overwrite=True)
                        out.append(ev)
                    si.on_wait = waits[:limit]
                out.append(inst)
            il[:] = out
    return ctr[0]


# ----------------------------------------------------------------------------
# device kernel emission
# ----------------------------------------------------------------------------

def _emit(nc, tile, mybir, lay):
    import concourse.bass as bass

    bf = mybir.dt.bfloat16
    f8 = mybir.dt.float8e4
    f32 = mybir.dt.float32
    DR = mybir.MatmulPerfMode.DoubleRow
    Exp = mybir.ActivationFunctionType.Exp
    ng = lay["ng"]
    NE = max(1, len(lay["vste_cases"]))

    f16 = mybir.dt.float16
    hs16_d = nc.dram_tensor("hs16", [D, S], f16, kind="ExternalInput")
    hsk16_d = nc.dram_tensor("hsk16", [D, KCOLS], f16, kind="ExternalInput")
    wq16_d = nc.dram_tensor("Wq16", [D, D], f16, kind="ExternalInput")
    wk16_d = nc.dram_tensor("Wk16", [D, D], f16, kind="ExternalInput")
    wv16_d = nc.dram_tensor("Wv16", [D, D], f16, kind="ExternalInput")
    bq8_d = nc.dram_tensor("bq8", [1, 1024], f8, kind="ExternalInput")
    bv16_d = nc.dram_tensor("bv16", [1, D], f16, kind="ExternalInput")
    i8_d = nc.dram_tensor("I8", [50, 2 * KEYS], f8, kind="ExternalInput")
    cz16_d = nc.dram_tensor("cz16", [1, 2048], bf, kind="ExternalInput")
    lb8_d = nc.dram_tensor("lb8", [NH, ng, 50, 2048], f8, kind="ExternalInput")
    out_d = nc.dram_tensor("out_s", [S, D], f32, kind="ExternalOutput")
    DEBUG = int(os.environ.get("BEIT_DEBUG", "0"))
    if DEBUG:
        dbgq_d = nc.dram_tensor("dbg_q", [128, 6 * SP8], f8 if QK8 else bf, kind="ExternalOutput")
        dbgk_d = nc.dram_tensor("dbg_k", [128, 6 * SP8], f8 if QK8 else bf, kind="ExternalOutput")
        dbga_d = nc.dram_tensor("dbg_a", [KEYS, 1024], bf, kind="ExternalOutput")
        dbgv_d = nc.dram_tensor("dbg_v", [KEYS, NPAIR * NH * 65], bf, kind="ExternalOutput")

    def ap3(sl, s1, n1, s2, n2):
        return bass.AP(tensor=sl.tensor, offset=sl.offset,
                       ap=[list(sl.ap[0]), [s1, n1], [s2, n2]])

    def slot_col(s):
        return 65 * s if s < 7 else (512 + 65 * (s - 7) if s < 14
                                     else 1024 + 65 * (s - 14))

    av_banks = [(0, 455), (512, 455), (1024, AVW - 1024)]

    with tile.TileContext(nc) as tc, ExitStack() as ctx:
        consts = ctx.enter_context(tc.tile_pool(name="consts", bufs=1))
        persist = ctx.enter_context(tc.tile_pool(name="persist", bufs=1))

        c64 = consts.tile([1, 1024], f8, tag="c64", name="c64")
        nc.vector.memset(c64[:, :], 1.0 / 64.0)
        o8c = consts.tile([1, 1024], f8, tag="o8c", name="o8c")
        nc.vector.memset(o8c[:, 0:512], 1.0 / 64.0)
        nc.vector.memset(o8c[:, 512:1024], 0.0)
        bq8 = consts.tile([1, 1024], f8, tag="bq8", name="bq8")
        nc.gpsimd.dma_start(out=bq8[:, :], in_=bq8_d[:, :])
        bv16 = consts.tile([1, D], f16, tag="bv16", name="bv16")
        nc.gpsimd.dma_start(out=bv16[:, :], in_=bv16_d[:, :])
        ones16 = consts.tile([1, 128], f16, tag="ones16", name="ones16")
        nc.vector.memset(ones16[:, :], 1.0)
        i8sb = consts.tile([50, 2 * KEYS], f8, tag="i8", name="i8")
        nc.gpsimd.dma_start(out=i8sb[:, :], in_=i8_d[:, :])

        qkdt = f8 if QK8 else bf
        qT8 = persist.tile([128, 6 * SP8], qkdt, tag="qT8", name="qT8")
        kT8 = persist.tile([128, 6 * SP8], qkdt, tag="kT8", name="kT8")
        for t in range(6):
            nc.vector.memset(qT8[:, t * SP8 + S:(t + 1) * SP8], 0.0)
            nc.vector.memset(kT8[:, t * SP8 + KCOLS:(t + 1) * SP8], 0.0)
        vst = persist.tile([KEYS, NPAIR * NH * 65], bf, tag="vst", name="vst")
        vst4 = vst[:, :].rearrange("a (p h e) -> a p h e", p=NPAIR, h=NH)
        nc.vector.memset(vst4[:, :, :, 64:65], 1.0)
        vste = persist.tile([KEYS, NE * NH * 65], bf, tag="vste", name="vste")
        nc.gpsimd.memset(vste[:, :], 0.0)
        vste4 = vste[:, :].rearrange("a (e h c) -> a e h c", e=NE, h=NH)
        def bcast49(dram_sl, inner):
            # DRAM source broadcast across 49 partitions
            return bass.AP(tensor=dram_sl.tensor, offset=dram_sl.offset,
                           ap=[[0, 49]] + inner)
        outS = persist.tile([128, (NSLOT + 1) * D], f32, tag="outS", name="outS")

        # ---------------- phase A: projections ----------------
        with tc.tile_pool(name="phA", bufs=1) as phA, \
             tc.tile_pool(name="pp", bufs=3, space="PSUM") as pp, \
             tc.tile_pool(name="ppv", bufs=2, space="PSUM") as ppv:
            hs16, hsk16 = [], []
            w_sb = {"q": [], "k": [], "v": []}
            for t in range(6):
                ht = phA.tile([128, S], f16, tag=f"hs16_{t}", name=f"hs16_{t}")
                nc.sync.dma_start(out=ht[:, :], in_=hs16_d[t * 128:(t + 1) * 128, :])
                hs16.append(ht)
                wt = phA.tile([128, D], f16, tag=f"wq16_{t}", name=f"wq16_{t}")
                nc.sync.dma_start(out=wt[:, :], in_=wq16_d[t * 128:(t + 1) * 128, :])
                w_sb["q"].append(wt)
            for t in range(6):
                ht = phA.tile([128, KCOLS], f16, tag=f"hsk16_{t}", name=f"hsk16_{t}")
                nc.gpsimd.dma_start(out=ht[:, :], in_=hsk16_d[t * 128:(t + 1) * 128, :])
                hsk16.append(ht)
            for nm, dram in (("k", wk16_d), ("v", wv16_d)):
                for t in range(6):
                    wt = phA.tile([128, D], f16, tag=f"w{nm}16_{t}", name=f"w{nm}16_{t}")
                    nc.gpsimd.dma_start(out=wt[:, :],
                                        in_=dram[t * 128:(t + 1) * 128, :])
                    w_sb[nm].append(wt)

            # qT8 / kT8 projections (fp16 matmuls, fp8 store): psum [128, chunk]
            qchunks = [(0, 512), (512, 512), (1024, 512), (1536, S - 1536)]
            kchunks = [(0, 512), (512, 512), (1024, 512), (1536, KCOLS - 1536)]
            for name, hsrc, chunks, dst, scale in (
                    ("q", hs16, qchunks, qT8, 0.5),
                    ("k", hsk16, kchunks, kT8, 0.25)):
                for dt in range(6):
                    for (c0, cw) in chunks:
                        ps = pp.tile([128, 512], f32, tag="pq", name="pq")
                        for kt in range(6):
                            nc.tensor.matmul(
                                ps[:, 0:cw],
                                lhsT=w_sb[name][kt][:, dt * 128:(dt + 1) * 128],
                                rhs=hsrc[kt][:, c0:c0 + cw],
                                start=(kt == 0),
                                stop=(kt == 5 and name == "k"))
                        if name == "q":
                            s0 = 0
                            while s0 < cw:
                                sw = min(256, cw - s0)
                                lhsT = ap3(bq8[0:1, dt * 128:dt * 128 + 1],
                                           D - dt * 128, 2, 1, 128)
                                rhs = ap3(c64[0:1, 0:1], 512, 2, 1, sw)
                                nc.tensor.matmul(ps[:, s0:s0 + sw], lhsT=lhsT,
                                                 rhs=rhs, start=False,
                                                 stop=(s0 + sw >= cw),
                                                 perf_mode=DR)
                                s0 += sw
                        nc.vector.tensor_scalar_mul(
                            dst[:, dt * SP8 + c0:dt * SP8 + c0 + cw],
                            ps[:, 0:cw], scale)

            # V projection per pair (pair-major hs incl. the cls-dup column,
            # so row 98 of each pair slice is v_cls): psum [99 tokens, 768]
            ecase = {pc: e for e, pc in enumerate(lay["vste_cases"])}
            for p in range(NPAIR):
                c0 = KEYS * p
                ps = ppv.tile([128, D], f32, tag="pv", name="pv")
                for (h0, hw_) in ((0, 512), (512, 256)):
                    for kt in range(6):
                        nc.tensor.matmul(
                            ps[0:KEYS, h0:h0 + hw_],
                            lhsT=hsk16[kt][:, c0:c0 + KEYS],
                            rhs=w_sb["v"][kt][:, h0:h0 + hw_],
                            start=(kt == 0), stop=False)
                    nc.tensor.matmul(ps[0:KEYS, h0:h0 + hw_],
                                     lhsT=ones16[0:1, 0:KEYS],
                                     rhs=bv16[0:1, h0:h0 + hw_],
                                     start=False, stop=True)
                src = ps[0:KEYS, :].rearrange("a (h e) -> a h e", h=NH)
                nc.gpsimd.tensor_copy(vst4[0:KEYS, p, :, 0:64], src)
                for half in range(2):
                    if (p, half) not in ecase:
                        continue
                    e = ecase[(p, half)]
                    nc.gpsimd.tensor_copy(vste4[0:98, e, :, 0:64],
                                          ps[0:98, :].rearrange(
                                              "a (h e) -> a h e", h=NH))
                    if half == 0:
                        # zero the inactive upper half, rider=1 on lower
                        nc.sync.dma_start(out=vste4[49:98, e, :, 0:65],
                                          in_=bcast49(cz16_d[0:1, 0:1],
                                                      [[65, NH], [1, 65]]))
                        nc.gpsimd.memset(vste4[0:49, e, :, 64:65], 1.0)
                    else:
                        # zero the inactive lower half, rider=1 on upper
                        nc.gpsimd.memset(vste4[0:49, e, :, 0:65], 0.0)
                        nc.sync.dma_start(
                            out=vste4[49:98, e, :, 64:65],
                            in_=bcast49(cz16_d[0:1, 1024:1025],
                                        [[1, NH], [1, 1]]))

        # ---------------- phase B: block-sparse attention per head ----------
        with tc.tile_pool(name="scps", bufs=2, space="PSUM") as scps, \
             tc.tile_pool(name="avps", bufs=1, space="PSUM") as avps, \
             tc.tile_pool(name="ab", bufs=3) as ab, \
             tc.tile_pool(name="lbp", bufs=3) as lbp, \
             tc.tile_pool(name="nrm", bufs=2) as nrm:

            def emit_openers(avt):
                # init every used AV psum byte to a tiny value ((1/64)^2 * 2)
                for (b0, bw) in av_banks:
                    first = True
                    c = 0
                    while c < bw:
                        take = min(256, bw - c)
                        nc.tensor.matmul(
                            avt[:, b0 + c:b0 + c + take],
                            lhsT=ap3(c64[0:1, 0:1], 512, 2, 1, 128),
                            rhs=ap3(c64[0:1, 0:1], 512, 2, 1, take),
                            start=first, stop=False, perf_mode=DR)
                        first = False
                        c += take

            def emit_av(h, g, aT, avt):
                for (kind, pe, oc, w, qp0, cb) in lay["av_pieces"][g]:
                    if kind == "vst":
                        rhs = vst4[0:KEYS, pe, h, 0:65]
                    else:
                        rhs = vste4[0:KEYS, pe, h, 0:65]
                    nc.tensor.matmul(
                        avt[qp0:qp0 + w, cb:cb + 65],
                        lhsT=aT[0:KEYS, oc:oc + w], rhs=rhs,
                        start=False, stop=False)

            def emit_head_tail(h, avt):
                # 1-col closers: end each bank's group on all 128 partitions
                for (b0, bw) in av_banks:
                    nc.tensor.matmul(avt[:, b0:b0 + 1],
                                     lhsT=ap3(c64[0:1, 0:1], 512, 2, 1, 128),
                                     rhs=ap3(c64[0:1, 0:1], 512, 2, 1, 1),
                                     start=False, stop=True, perf_mode=DR)
                rcol = nrm.tile([128, 17], f32, tag="rcol", name="rcol")
                for (i0, i1, base, n) in ((0, 7, 64, 7),
                                          (7, 14, 512 + 64, 7),
                                          (14, 17, 1024 + 64, 3)):
                    d0 = avt[:, base:base + 1]
                    nc.vector.reciprocal(
                        rcol[:, i0:i1],
                        bass.AP(tensor=d0.tensor, offset=d0.offset,
                                ap=[list(d0.ap[0]), [65, n]]))
                for s in range(NSLOT):
                    eng = nc.vector if s % 2 == 0 else nc.gpsimd
                    eng.tensor_scalar_mul(
                        outS[:, s * D + h * DH:s * D + h * DH + DH],
                        avt[:, slot_col(s):slot_col(s) + 64],
                        rcol[:, s:s + 1])
                nc.vector.tensor_scalar_mul(
                    outS[0:1, NSLOT * D + h * DH:NSLOT * D + h * DH + DH],
                    avt[0:1, CLSB:CLSB + 64], rcol[0:1, 16:17])

            pending = None
            avt_by_h = {}
            for h in range(NH):
                dt, r0 = h // 2, 64 * (h % 2)
                avt = avps.tile([128, 1536], f32, tag="avt", name="avt")
                avt_by_h[h] = avt
                for g in range(ng):
                    gw = lay["gocc"][g]
                    sc = scps.tile([128, 1024], f32, tag="sc", name="sc")
                    lb = lbp.tile([50, 2048], f8, tag="lb", name="lb")
                    nc.sync.dma_start(out=lb[:, :], in_=lb8_d[h, g])
                    for (p, rc, rw, oc, st) in lay["score_pieces"][g]:
                        if QK8:
                            lhsT = ap3(kT8[r0:r0 + 64, dt * SP8 + 99 * p:dt * SP8 + 99 * p + 1],
                                       KZ - 99 * p, 2, 1, KEYS)
                            rhs = ap3(qT8[r0:r0 + 64, dt * SP8 + rc:dt * SP8 + rc + 1],
                                      QZ - rc, 2, 1, rw)
                            nc.tensor.matmul(sc[0:KEYS, oc:oc + rw], lhsT=lhsT,
                                             rhs=rhs, start=st, stop=False,
                                             perf_mode=DR)
                        else:
                            nc.tensor.matmul(
                                sc[0:KEYS, oc:oc + rw],
                                lhsT=kT8[r0:r0 + 64, dt * SP8 + 99 * p:dt * SP8 + 99 * p + KEYS],
                                rhs=qT8[r0:r0 + 64, dt * SP8 + rc:dt * SP8 + rc + rw],
                                start=st, stop=False)
                    for (bc0, bw, sp) in lay["bias_pieces"][g]:
                        lhsT = ap3(i8sb[0:50, 0:1], KEYS, 2, 1, KEYS)
                        rhs = ap3(lb[0:50, bc0:bc0 + 1], 1024, 2, 1, bw)
                        nc.tensor.matmul(sc[0:KEYS, bc0:bc0 + bw], lhsT=lhsT,
                                         rhs=rhs, start=False, stop=sp,
                                         perf_mode=DR)
                    aT = ab.tile([KEYS, 1024], bf, tag="aT", name="aT")
                    nc.scalar.activation(aT[:, 0:gw], sc[0:KEYS, 0:gw], Exp)
                    if DEBUG and h == 0 and g == 0:
                        nc.sync.dma_start(out=dbga_d[:, 0:gw], in_=aT[0:KEYS, 0:gw])
                    if pending is not None:
                        ph, pg, paT = pending
                        if pg == 0:
                            emit_openers(avt_by_h[ph])
                        emit_av(ph, pg, paT, avt_by_h[ph])
                        if pg == ng - 1:
                            emit_head_tail(ph, avt_by_h.pop(ph))
                    pending = (h, g, aT)
            ph, pg, paT = pending
            if pg == 0:
                emit_openers(avt_by_h[ph])
            emit_av(ph, pg, paT, avt_by_h[ph])
            emit_head_tail(ph, avt_by_h.pop(ph))

            if DEBUG:
                nc.sync.dma_start(out=dbgq_d[:, :], in_=qT8[:, :])
                nc.sync.dma_start(out=dbgk_d[:, :], in_=kT8[:, :])
                nc.sync.dma_start(out=dbgv_d[:, :], in_=vst[:, :])
            # output DMA per slot half (+ cls token row)
            for s in range(NSLOT):
                t0 = 1 + 98 * s
                nc.gpsimd.dma_start(out=out_d[t0:t0 + 49, :],
                                    in_=outS[0:49, s * D:(s + 1) * D])
                nc.gpsimd.dma_start(out=out_d[t0 + 49:t0 + 98, :],
                                    in_=outS[64:113, s * D:(s + 1) * D])
            nc.gpsimd.dma_start(out=out_d[0:1, :],
                                in_=outS[0:1, NSLOT * D:(NSLOT + 1) * D])

    _split_excess_waits(nc, mybir, limit=1)
    return nc


# ----------------------------------------------------------------------------
# host-side input prep
# ----------------------------------------------------------------------------

def _prepare(hidden_states, Wq, bq, Wk, Wv, bv, rel_table, rel_pos_index, rand_idx):
    import ml_dtypes

    import concourse.bass as bass
    import concourse.tile as tile
    from concourse import mybir

    _patch_tile_drain()
    f8np = ml_dtypes.float8_e4m3

    hidden_states = np.asarray(hidden_states, np.float32)
    Wq = np.asarray(Wq, np.float32)
    Wk = np.asarray(Wk, np.float32)
    Wv = np.asarray(Wv, np.float32)
    bq = np.asarray(bq, np.float32)
    bv = np.asarray(bv, np.float32)
    rel_table = np.asarray(rel_table, np.float32)
    rel_pos_index = np.asarray(rel_pos_index)
    rand_idx = np.asarray(rand_idx)

    lay = _build_layout(rand_idx)
    lb8 = _build_bias8(lay, rel_table, rel_pos_index, f8np)
    f16np = np.float16

    i8 = np.zeros((50, 2, KEYS), np.float32)
    for p in range(49):
        i8[p, 0, p] = 1.0 / 16.0
        i8[p, 1, 49 + p] = 1.0 / 16.0
    i8[49, 1, 98] = 1.0 / 16.0

    shared = {
        "Wq16": Wq.astype(f16np), "Wk16": Wk.astype(f16np),
        "Wv16": Wv.astype(f16np),
        "bq8": np.concatenate([bq * 64.0, np.zeros(256, np.float32)]
                              ).reshape(1, 1024).astype(f8np),
        "bv16": bv.reshape(1, D).astype(f16np),
        "I8": i8.reshape(50, 2 * KEYS).astype(f8np),
        "lb8": lb8,
        "cz16": np.concatenate([np.zeros(1024, np.float32),
                                np.ones(1024, np.float32)]
                               ).reshape(1, 2048).astype(ml_dtypes.bfloat16),
    }

    # pair-major token order for the k/v projections (cls duplicated per pair)
    korder = np.empty(KCOLS, np.int64)
    for p in range(NPAIR):
        korder[99 * p:99 * p + 98] = 1 + 98 * p + np.arange(98)
        korder[99 * p + 98] = 0

    in_maps = []
    for b in range(B):
        hsT = hidden_states[b].T  # [768, S]
        m = dict(shared)
        m["hs16"] = np.ascontiguousarray(hsT).astype(f16np)
        m["hsk16"] = np.ascontiguousarray(hsT[:, korder]).astype(f16np)
        in_maps.append(m)

    nc = bass.Bass()
    _emit(nc, tile, mybir, lay)
    return nc, in_maps


# ----------------------------------------------------------------------------
# optional PJRT repeat-bench (unused by default; kept from v1)
# ----------------------------------------------------------------------------

def _bench_pjrt(nc, in_maps, n_cores, iters=20, warmup=3):
    import time

    import jax
    from jax.sharding import Mesh, PartitionSpec
    from jax.experimental.shard_map import shard_map

    from concourse import mybir
    from concourse.bass2jax import (_bass_exec_p, install_neuronx_cc_hook,
                                    partition_id_tensor)

    install_neuronx_cc_hook()
    partition_name = nc.partition_id_tensor.name if nc.partition_id_tensor else None
    in_names, out_names, out_avals, zero_outs = [], [], [], []
    for alloc in nc.m.functions[0].allocations:
        if not isinstance(alloc, mybir.MemoryLocationSet):
            continue
        name = alloc.memorylocations[0].name
        if alloc.kind == "ExternalInput":
            if name != partition_name:
                in_names.append(name)
        elif alloc.kind == "ExternalOutput":
            shape = tuple(alloc.tensor_shape)
            dtype = mybir.dt.np(alloc.dtype)
            out_names.append(name)
            out_avals.append(jax.core.ShapedArray(shape, dtype))
            zero_outs.append(np.zeros(shape, dtype))
    n_params = len(in_names)
    all_in_names = in_names + out_names + ([partition_name] if partition_name else [])

    def _body(*args):
        operands = list(args)
        if partition_name is not None:
            operands.append(partition_id_tensor())
        return tuple(_bass_exec_p.bind(
            *operands,
            out_avals=tuple(out_avals),
            in_names=tuple(all_in_names),
            out_names=tuple(out_names),
            lowering_input_output_aliases=(),
            sim_require_finite=True,
            sim_require_nnan=True,
            nc=nc,
        ))

    devices = jax.devices()[:n_cores]
    mesh = Mesh(np.asarray(devices), ("core",))
    n_outs = len(out_names)
    sharded = jax.jit(
        shard_map(_body, mesh=mesh,
                  in_specs=(PartitionSpec("core"),) * (n_params + n_outs),
                  out_specs=(PartitionSpec("core"),) * n_outs,
                  check_rep=False),
        keep_unused=True,
    )
    per_core = [[np.asarray(m[name]) for name in in_names] for m in in_maps]
    concat_in = [np.concatenate([per_core[c][i] for c in range(n_cores)], axis=0)
                 for i in range(n_params)]
    concat_zeros = [np.zeros((n_cores * z.shape[0], *z.shape[1:]), z.dtype)
                    for z in zero_outs]
    dev_in = [jax.device_put(a) for a in concat_in + concat_zeros]
    out = sharded(*dev_in)
    jax.block_until_ready(out)
    for _ in range(warmup):
        out = sharded(*dev_in)
    jax.block_until_ready(out)
    t0 = time.perf_counter()
    for _ in range(iters):
        out = sharded(*dev_in)
    jax.block_until_ready(out)
    dt = (time.perf_counter() - t0) / iters
    results = [
        {name: np.asarray(out[i]).reshape(n_cores, *out_avals[i].shape)[c]
         for i, name in enumerate(out_names)}
        for c in range(n_cores)
    ]
    return int(dt * 1e9), results


# ----------------------------------------------------------------------------
# public entry point
# ----------------------------------------------------------------------------

def kernel(hidden_states, Wq, bq, Wk, Wv, bv, rel_table, rel_pos_index, rand_idx):
    from concourse.bass_utils import run_bass_kernel_spmd

    nc, in_maps = _prepare(hidden_states, Wq, bq, Wk, Wv, bv,
                           rel_table, rel_pos_index, rand_idx)

    kernel.last_nc = nc
    kernel.last_in_maps = in_maps
    bench_iters = int(os.environ.get("BEIT_BENCH", "0"))
    if bench_iters > 0:
        per_iter_ns, results = _bench_pjrt(nc, in_maps, N_CORES, iters=bench_iters)
        kernel.last_exec_time_ns = per_iter_ns
    else:
        res = run_bass_kernel_spmd(nc, in_maps, core_ids=list(range(N_CORES)))
        results = res.results

    out = np.empty((B, S, D), np.float32)
    for b in range(B):
        out[b] = results[b]["out_s"]
    return out


# revision 32
# speedup vs baseline: 1.0214x; 1.0022x over previous
"""BeitSelfAttention block-sparse attention kernel for 8 Trainium2 NeuronCores.

Strategy (data-parallel over batch, B=8 -> one batch element per core):
  - fp8e4 DoubleRow matmuls for QKV projections and block-sparse scores
    (two 128-row k-tiles per pass at 0.5 cycles/row).
  - The relative-position bias AND the block-sparsity mask are host-packed
    into one fp8 table (16x scale) and added into the score PSUM by an
    identity DoubleRow matmul; gather multiplicity (rand/local block
    collisions) is realized by a few extra AV matmuls against half-masked V
    copies, so no per-element multiply is needed on DVE at all.
  - The cls KEY rides as a 99th score row per key-pair tile (designated to
    one pair per query via the bias mask); the cls QUERY is packed column 0.
  - AV is computed transposed: out[q, dh] psum tiles [128 queries, 65] with a
    ones-rider column in V accumulating the softmax denominator per query
    IN THE FREE DIM, so normalization is a per-partition reciprocal +
    scaled copy (no cross-partition broadcast, no DRAM round trip).
  - Output written as [S, 768] fp32 - already the final layout.
"""

import os
from contextlib import ExitStack

import numpy as np

NCLS, BS, NBLK, NPAIR, NH, DH = 1, 49, 32, 16, 12, 64
B, S, D = 8, 1569, 768
NTOK = S - NCLS  # 1568
N_CORES = 8
QK8 = int(__import__('os').environ.get('QK8', '1')) != 0
KEYS = 99            # 98 pair keys + 1 cls row
NSLOT = 16           # AV psum slots (2 blocks each, partitions 0-48 / 64-112)
CLSB = 1024 + 2 * 65  # cls-query corner col in the AV psum tile (bank 2)
AVW = CLSB + 65      # used width of the AV psum tile
SP8 = 1856           # per-dt stride of qT8/kT8 (zero strip beyond data)
KCOLS = NPAIR * KEYS  # 1584 pair-major key columns
QZ = 1576            # qT8 ktile1 landing col (zeros)
KZ = 1584            # kT8 ktile1 landing col (zeros)


# ----------------------------------------------------------------------------
# host-side layout
# ----------------------------------------------------------------------------

def _slot_target(qtok):
    """Map a query token to its AV psum target: (partition0, col_base).
    Slot s holds block 2s at partitions 0-48 and block 2s+1 at 64-112;
    the cls query (token 0) lives at partition 0 of the cls corner."""
    if qtok == 0:
        return 0, CLSB
    qb = (qtok - 1) // BS
    s = qb // 2
    cb = 65 * s if s < 7 else (512 + 65 * (s - 7) if s < 14 else 1024 + 65 * (s - 14))
    return 64 * (qb % 2) + (qtok - 1 - BS * qb), cb


def _build_layout(rand_idx):
    rand_idx = np.asarray(rand_idx)
    mult = np.zeros((NBLK, NBLK), np.int32)
    for m in range(NBLK):
        for o in (-1, 0, 1):
            mult[m, (m + o) % NBLK] += 1
        for r in rand_idx[m]:
            mult[m, int(r)] += 1

    # pack attending query columns per key-pair into banks of 512 (groups of
    # 1024). Block units (the cls col, or a 49-token block) never straddle a
    # bank boundary: the packing pads to the bank edge instead, so every AV
    # piece starts at an aligned psum partition (0 or 64). Pad columns carry
    # no scores; the bias matmul assigns them the -448 mask.
    segs = []
    gcol = 0
    for p in range(NPAIR):
        att = sorted(set(np.nonzero(mult[:, 2 * p])[0])
                     | set(np.nonzero(mult[:, 2 * p + 1])[0]))
        units = [(0, 1)] + [(1 + BS * m, BS) for m in att]
        cur = None
        prev_end = None
        for (uc, uw) in units:
            if 512 - (gcol % 512) < uw:
                gcol += 512 - (gcol % 512)  # pad to bank edge
                cur = None
            if cur is None or cur["bank"] != gcol // 512:
                cur = {"p": p, "runs": [], "bank": gcol // 512,
                       "off": gcol % 512}
                segs.append(cur)
                prev_end = None
            if prev_end == uc:
                rc0, rw0 = cur["runs"][-1]
                cur["runs"][-1] = (rc0, rw0 + uw)
            else:
                cur["runs"].append((uc, uw))
            prev_end = uc + uw
            gcol += uw
        cur = None  # next pair starts a new segment

    nbank = (gcol + 511) // 512
    ng = (nbank + 1) // 2
    for sg in segs:
        sg["g"] = sg["bank"] // 2
        sg["goff"] = (sg["bank"] % 2) * 512 + sg["off"]
    # group occupancy (incl. pad columns): all banks full except the last
    last_bank_fill = gcol - (nbank - 1) * 512
    gocc = []
    for g in range(ng):
        b0, b1 = 2 * g, 2 * g + 1
        occ = 0
        for b in (b0, b1):
            if b < nbank - 1:
                occ += 512
            elif b == nbank - 1:
                occ += last_bank_fill
        gocc.append(occ)

    # per-group score pieces (runs split to <=256), start flag per 512-region
    score_pieces = [[] for _ in range(ng)]
    bank_started = [False] * nbank
    for sg in segs:
        oc = 0
        for (rc, rw) in sg["runs"]:
            c, w = rc, rw
            while w > 0:
                take = min(w, 256)
                st = not bank_started[sg["bank"]]
                bank_started[sg["bank"]] = True
                score_pieces[sg["g"]].append(
                    (sg["p"], c, take, sg["goff"] + oc, st))
                oc += take
                c += take
                w -= take

    # bias pieces per group: cover each bank's occupancy in <=256 chunks,
    # split at the pad watermark (scores wrote [0, used); pads [used, bw) are
    # still pending-zero and must be covered by their own assign piece);
    # last chunk per bank carries stop
    bank_used = [0] * nbank
    for sg in segs:
        w = sum(rw for (_, rw) in sg["runs"])
        bank_used[sg["bank"]] = max(bank_used[sg["bank"]], sg["off"] + w)
    bias_pieces = [[] for _ in range(ng)]
    for g in range(ng):
        for half in range(2):
            b = 2 * g + half
            if b >= nbank:
                continue
            bw = 512 if b < nbank - 1 else last_bank_fill
            used = bank_used[b]
            c = 0
            while c < bw:
                lim = used if c < used else bw
                take = min(256, lim - c)
                bias_pieces[g].append(
                    (half * 512 + c, take, c + take >= bw))
                c += take

    # AV pieces per group: runs split at block units -> aligned partitions
    av_pieces = [[] for _ in range(ng)]
    for sg in segs:
        oc = 0
        for (rc, rw) in sg["runs"]:
            c, w = rc, rw
            while w > 0:
                take = 1 if c == 0 else min(w, BS - (c - 1) % BS)
                qp0, cb = _slot_target(c)
                av_pieces[sg["g"]].append(
                    ("vst", sg["p"], sg["goff"] + oc, take, qp0, cb))
                oc += take
                c += take
                w -= take

    # per-group column -> (qtok, pair) maps (qtok -1 = pad), cls designation
    lb_cols = []
    for g in range(ng):
        qtok = np.full(1024, -1, np.int64)
        pair = np.zeros(1024, np.int64)
        lb_cols.append((qtok, pair))
    cls_seen = np.zeros(S, bool)
    cls_des = np.zeros((ng, 1024), bool)
    for sg in segs:
        qtok, pair = lb_cols[sg["g"]]
        oc = sg["goff"]
        for (rc, rw) in sg["runs"]:
            qtok[oc:oc + rw] = np.arange(rc, rc + rw)
            pair[oc:oc + rw] = sg["p"]
            fresh = ~cls_seen[rc:rc + rw]
            cls_des[sg["g"], oc:oc + rw] = fresh
            cls_seen[rc:rc + rw] = True
            oc += rw

    # multiplicity extras: (qb, kb) with mult >= 2 -> (m-1) extra AV matmuls
    # against a half-masked V copy (vste slot per distinct (pair, half))
    vste_cases = []      # (pair, half)
    col_of = {}
    for sg in segs:
        oc = sg["goff"]
        for (rc, rw) in sg["runs"]:
            for i in range(rw):
                col_of[(sg["p"], rc + i)] = (sg["g"], oc + i)
            oc += rw
    for qb in range(NBLK):
        for kb in range(NBLK):
            m = int(mult[qb, kb])
            if m < 2:
                continue
            p, half = kb // 2, kb % 2
            if (p, half) not in vste_cases:
                vste_cases.append((p, half))
            e = vste_cases.index((p, half))
            t0 = 1 + BS * qb
            g, oc = col_of[(p, t0)]
            qp0, cb = _slot_target(t0)
            for _ in range(m - 1):
                av_pieces[g].append(("vste", e, oc, BS, qp0, cb))
    return {"segs": segs, "mult": mult, "ng": ng, "gocc": gocc,
            "nbank": nbank, "score_pieces": score_pieces,
            "bias_pieces": bias_pieces, "av_pieces": av_pieces,
            "lb_cols": lb_cols, "cls_des": cls_des,
            "vste_cases": vste_cases}


def _build_bias8(lay, rel_table, rel_pos_index, f8np):
    """lb8 [NH, ng, 50, 2048]: rows (p, i) -> key 49i+p (cls at (49,1)),
    values 16*bias, -240 where masked."""
    ng = lay["ng"]
    mult = lay["mult"]
    MASK = -240.0
    lb = np.full((NH, ng, 50, 2, 1024), MASK, np.float32)
    for sg in lay["segs"]:
        g = sg["g"]
        p = sg["p"]
        ktok = 1 + 98 * p + np.arange(98)          # [98]
        kblk = 2 * p + (np.arange(98) // BS)
        oc = sg["goff"]
        for (rc, rw) in sg["runs"]:
            qtok = np.arange(rc, rc + rw)
            qblk = np.maximum(qtok - 1, 0) // BS
            att = (mult[qblk][:, kblk] > 0) | (qtok == 0)[:, None]  # [rw, 98]
            idx = rel_pos_index[qtok[:, None], ktok[None, :]]       # [rw, 98]
            val = 16.0 * rel_table[idx]                             # [rw,98,NH]
            val = np.where(att[:, :, None], np.clip(val, -200, 200), MASK)
            v = val.transpose(2, 1, 0)                              # [NH,98,rw]
            lb[:, g, 0:49, 0, oc:oc + rw] = v[:, 0:49]
            lb[:, g, 0:49, 1, oc:oc + rw] = v[:, 49:98]
            # cls row: designated pair only
            des = lay["cls_des"][g, oc:oc + rw]
            cidx = rel_pos_index[qtok, 0]
            cval = np.clip(16.0 * rel_table[cidx], -200, 200)       # [rw, NH]
            cv = np.where(des[:, None], cval, MASK).T               # [NH, rw]
            lb[:, g, 49, 1, oc:oc + rw] = cv
            oc += rw
    return lb.reshape(NH, ng, 50, 2048).astype(f8np)


# ----------------------------------------------------------------------------
# walrus workaround: split the TileContext tail drain's sem waits
# ----------------------------------------------------------------------------

def _patch_tile_drain():
    import concourse.tile as tile
    from concourse.vector_clock import ScopedClock, VectorClock

    if getattr(tile.TileContext, "_beit_drain_patch", False):
        return

    def _drain_and_barrier(self, tick_clock, wait_clock):
        gc_vec = tick_clock.global_clock
        n = len(gc_vec)
        nonzero = [i for i in range(n) if gc_vec[i] > 0] or [0]
        for i in range(0, len(nonzero), 1):
            chunk = set(nonzero[i:i + 1])
            vec = VectorClock([gc_vec[j] if j in chunk else 0 for j in range(n)])
            drain_inst = self.nc.sync.drain()
            wait_clock.add_sem_waits(drain_inst.ins, ScopedClock({None: vec}))
        self.nc.all_engine_barrier()
        assert self.sems is not None
        popped = self.nc._tile_sem_poison_stack.pop()
        assert popped is self._sem_poison
        self.nc.clear_and_free_semaphores(list(self.sems.allocated().values()))
        self.nc.all_engine_barrier()

    tile.TileContext._drain_and_barrier = _drain_and_barrier
    tile.TileContext._beit_drain_patch = True


def _split_excess_waits(nc, mybir, limit=1):
    """This walrus build allows very few sem waits per instruction; move the
    excess onto EventSemaphore carrier instructions inserted just before."""
    ctr = [0]
    for f in nc.m.functions:
        for bb in f.blocks:
            il = bb.instructions
            out = []
            for inst in il:
                si = inst.sync_info
                if si is not None and si.on_wait and len(si.on_wait) > limit:
                    waits = list(si.on_wait)
                    over = waits[limit:]
                    for j in range(0, len(over), limit):
                        ctr[0] += 1
                        ev = mybir.InstEventSemaphore(
                            name=f"WSPLIT-{ctr[0]}", ins=[], outs=[],
                            engine=inst.engine,
                            sync_info=mybir.SyncInfo(on_wait=over[j:j + limit],
                                                     on_update=[]),
                        )
                        nc.register_instruction(ev, overwrite=True)
                        out.append(ev)
                    si.on_wait = waits[:limit]
                out.append(inst)
            il[:] = out
    return ctr[0]


# ----------------------------------------------------------------------------
# device kernel emission
# ----------------------------------------------------------------------------

def _emit(nc, tile, mybir, lay):
    import concourse.bass as bass

    bf = mybir.dt.bfloat16
    f8 = mybir.dt.float8e4
    f32 = mybir.dt.float32
    DR = mybir.MatmulPerfMode.DoubleRow
    Exp = mybir.ActivationFunctionType.Exp
    ng = lay["ng"]
    NE = max(1, len(lay["vste_cases"]))

    f16 = mybir.dt.float16
    hs16_d = nc.dram_tensor("hs16", [D, S], f16, kind="ExternalInput")
    hsk16_d = nc.dram_tensor("hsk16", [D, KCOLS], f16, kind="ExternalInput")
    wq16_d = nc.dram_tensor("Wq16", [D, D], f16, kind="ExternalInput")
    wk16_d = nc.dram_tensor("Wk16", [D, D], f16, kind="ExternalInput")
    wv16_d = nc.dram_tensor("Wv16", [D, D], f16, kind="ExternalInput")
    bq8_d = nc.dram_tensor("bq8", [1, 1024], f8, kind="ExternalInput")
    bv16_d = nc.dram_tensor("bv16", [1, D], f16, kind="ExternalInput")
    i8_d = nc.dram_tensor("I8", [50, 2 * KEYS], f8, kind="ExternalInput")
    cz16_d = nc.dram_tensor("cz16", [1, 2048], bf, kind="ExternalInput")
    lb8_d = nc.dram_tensor("lb8", [NH, ng, 50, 2048], f8, kind="ExternalInput")
    out_d = nc.dram_tensor("out_s", [S, D], f32, kind="ExternalOutput")
    DEBUG = int(os.environ.get("BEIT_DEBUG", "0"))
    if DEBUG:
        dbgq_d = nc.dram_tensor("dbg_q", [128, 6 * SP8], f8 if QK8 else bf, kind="ExternalOutput")
        dbgk_d = nc.dram_tensor("dbg_k", [128, 6 * SP8], f8 if QK8 else bf, kind="ExternalOutput")
        dbga_d = nc.dram_tensor("dbg_a", [KEYS, 1024], bf, kind="ExternalOutput")
        dbgv_d = nc.dram_tensor("dbg_v", [KEYS, NPAIR * NH * 65], bf, kind="ExternalOutput")

    def ap3(sl, s1, n1, s2, n2):
        return bass.AP(tensor=sl.tensor, offset=sl.offset,
                       ap=[list(sl.ap[0]), [s1, n1], [s2, n2]])

    def slot_col(s):
        return 65 * s if s < 7 else (512 + 65 * (s - 7) if s < 14
                                     else 1024 + 65 * (s - 14))

    av_banks = [(0, 455), (512, 455), (1024, AVW - 1024)]

    with tile.TileContext(nc) as tc, ExitStack() as ctx:
        consts = ctx.enter_context(tc.tile_pool(name="consts", bufs=1))
        persist = ctx.enter_context(tc.tile_pool(name="persist", bufs=1))

        c64 = consts.tile([1, 1024], f8, tag="c64", name="c64")
        nc.vector.memset(c64[:, :], 1.0 / 64.0)
        o8c = consts.tile([1, 1024], f8, tag="o8c", name="o8c")
        nc.vector.memset(o8c[:, 0:512], 1.0 / 64.0)
        nc.vector.memset(o8c[:, 512:1024], 0.0)
        bq8 = consts.tile([1, 1024], f8, tag="bq8", name="bq8")
        nc.gpsimd.dma_start(out=bq8[:, :], in_=bq8_d[:, :])
        bv16 = consts.tile([1, D], f16, tag="bv16", name="bv16")
        nc.gpsimd.dma_start(out=bv16[:, :], in_=bv16_d[:, :])
        ones16 = consts.tile([1, 128], f16, tag="ones16", name="ones16")
        nc.vector.memset(ones16[:, :], 1.0)
        i8sb = consts.tile([50, 2 * KEYS], f8, tag="i8", name="i8")
        nc.gpsimd.dma_start(out=i8sb[:, :], in_=i8_d[:, :])

        qkdt = f8 if QK8 else bf
        qT8 = persist.tile([128, 6 * SP8], qkdt, tag="qT8", name="qT8")
        kT8 = persist.tile([128, 6 * SP8], qkdt, tag="kT8", name="kT8")
        for t in range(6):
            nc.vector.memset(qT8[:, t * SP8 + S:(t + 1) * SP8], 0.0)
            nc.vector.memset(kT8[:, t * SP8 + KCOLS:(t + 1) * SP8], 0.0)
        vst = persist.tile([KEYS, NPAIR * NH * 65], bf, tag="vst", name="vst")
        vst4 = vst[:, :].rearrange("a (p h e) -> a p h e", p=NPAIR, h=NH)
        nc.vector.memset(vst4[:, :, :, 64:65], 1.0)
        vste = persist.tile([KEYS, NE * NH * 65], bf, tag="vste", name="vste")
        nc.gpsimd.memset(vste[:, :], 0.0)
        vste4 = vste[:, :].rearrange("a (e h c) -> a e h c", e=NE, h=NH)
        def bcast49(dram_sl, inner):
            # DRAM source broadcast across 49 partitions
            return bass.AP(tensor=dram_sl.tensor, offset=dram_sl.offset,
                           ap=[[0, 49]] + inner)
        outS = persist.tile([128, (NSLOT + 1) * D], f32, tag="outS", name="outS")

        # ---------------- phase A: projections ----------------
        with tc.tile_pool(name="phA", bufs=1) as phA, \
             tc.tile_pool(name="pp", bufs=3, space="PSUM") as pp, \
             tc.tile_pool(name="ppv", bufs=2, space="PSUM") as ppv:
            hs16, hsk16 = [], []
            w_sb = {"q": [], "k": [], "v": []}
            for t in range(6):
                ht = phA.tile([128, S], f16, tag=f"hs16_{t}", name=f"hs16_{t}")
                nc.sync.dma_start(out=ht[:, :], in_=hs16_d[t * 128:(t + 1) * 128, :])
                hs16.append(ht)
                wt = phA.tile([128, D], f16, tag=f"wq16_{t}", name=f"wq16_{t}")
                nc.sync.dma_start(out=wt[:, :], in_=wq16_d[t * 128:(t + 1) * 128, :])
                w_sb["q"].append(wt)
            for t in range(6):
                ht = phA.tile([128, KCOLS], f16, tag=f"hsk16_{t}", name=f"hsk16_{t}")
                nc.gpsimd.dma_start(out=ht[:, :], in_=hsk16_d[t * 128:(t + 1) * 128, :])
                hsk16.append(ht)
            for nm, dram in (("k", wk16_d), ("v", wv16_d)):
                for t in range(6):
                    wt = phA.tile([128, D], f16, tag=f"w{nm}16_{t}", name=f"w{nm}16_{t}")
                    nc.gpsimd.dma_start(out=wt[:, :],
                                        in_=dram[t * 128:(t + 1) * 128, :])
                    w_sb[nm].append(wt)

            # qT8 / kT8 projections (fp16 matmuls, fp8 store): psum [128, chunk]
            qchunks = [(0, 512), (512, 512), (1024, 512), (1536, S - 1536)]
            kchunks = [(0, 512), (512, 512), (1024, 512), (1536, KCOLS - 1536)]
            for name, hsrc, chunks, dst, scale in (
                    ("q", hs16, qchunks, qT8, 0.5),
                    ("k", hsk16, kchunks, kT8, 0.25)):
                for dt in range(6):
                    for (c0, cw) in chunks:
                        ps = pp.tile([128, 512], f32, tag="pq", name="pq")
                        for kt in range(6):
                            nc.tensor.matmul(
                                ps[:, 0:cw],
                                lhsT=w_sb[name][kt][:, dt * 128:(dt + 1) * 128],
                                rhs=hsrc[kt][:, c0:c0 + cw],
                                start=(kt == 0),
                                stop=(kt == 5 and name == "k"))
                        if name == "q":
                            s0 = 0
                            while s0 < cw:
                                sw = min(256, cw - s0)
                                lhsT = ap3(bq8[0:1, dt * 128:dt * 128 + 1],
                                           D - dt * 128, 2, 1, 128)
                                rhs = ap3(c64[0:1, 0:1], 512, 2, 1, sw)
                                nc.tensor.matmul(ps[:, s0:s0 + sw], lhsT=lhsT,
                                                 rhs=rhs, start=False,
                                                 stop=(s0 + sw >= cw),
                                                 perf_mode=DR)
                                s0 += sw
                        nc.vector.tensor_scalar_mul(
                            dst[:, dt * SP8 + c0:dt * SP8 + c0 + cw],
                            ps[:, 0:cw], scale)

            # V projection per pair (pair-major hs incl. the cls-dup column,
            # so row 98 of each pair slice is v_cls): psum [99 tokens, 768]
            ecase = {pc: e for e, pc in enumerate(lay["vste_cases"])}
            for p in range(NPAIR):
                c0 = KEYS * p
                ps = ppv.tile([128, D], f32, tag="pv", name="pv")
                for (h0, hw_) in ((0, 512), (512, 256)):
                    for kt in range(6):
                        nc.tensor.matmul(
                            ps[0:KEYS, h0:h0 + hw_],
                            lhsT=hsk16[kt][:, c0:c0 + KEYS],
                            rhs=w_sb["v"][kt][:, h0:h0 + hw_],
                            start=(kt == 0), stop=False)
                    nc.tensor.matmul(ps[0:KEYS, h0:h0 + hw_],
                                     lhsT=ones16[0:1, 0:KEYS],
                                     rhs=bv16[0:1, h0:h0 + hw_],
                                     start=False, stop=True)
                src = ps[0:KEYS, :].rearrange("a (h e) -> a h e", h=NH)
                nc.gpsimd.tensor_copy(vst4[0:KEYS, p, :, 0:64], src)
                for half in range(2):
                    if (p, half) not in ecase:
                        continue
                    e = ecase[(p, half)]
                    nc.gpsimd.tensor_copy(vste4[0:98, e, :, 0:64],
                                          ps[0:98, :].rearrange(
                                              "a (h e) -> a h e", h=NH))
                    if half == 0:
                        # zero the inactive upper half, rider=1 on lower
                        nc.sync.dma_start(out=vste4[49:98, e, :, 0:65],
                                          in_=bcast49(cz16_d[0:1, 0:1],
                                                      [[65, NH], [1, 65]]))
                        nc.gpsimd.memset(vste4[0:49, e, :, 64:65], 1.0)
                    else:
                        # zero the inactive lower half, rider=1 on upper
                        nc.gpsimd.memset(vste4[0:49, e, :, 0:65], 0.0)
                        nc.sync.dma_start(
                            out=vste4[49:98, e, :, 64:65],
                            in_=bcast49(cz16_d[0:1, 1024:1025],
                                        [[1, NH], [1, 1]]))

        # ---------------- phase B: block-sparse attention per head ----------
        with tc.tile_pool(name="scps", bufs=2, space="PSUM") as scps, \
             tc.tile_pool(name="avps", bufs=1, space="PSUM") as avps, \
             tc.tile_pool(name="ab", bufs=4) as ab, \
             tc.tile_pool(name="lbp", bufs=4) as lbp, \
             tc.tile_pool(name="nrm", bufs=2) as nrm:

            def emit_openers(avt):
                # init every used AV psum byte to a tiny value ((1/64)^2 * 2)
                for (b0, bw) in av_banks:
                    first = True
                    c = 0
                    while c < bw:
                        take = min(256, bw - c)
                        nc.tensor.matmul(
                            avt[:, b0 + c:b0 + c + take],
                            lhsT=ap3(c64[0:1, 0:1], 512, 2, 1, 128),
                            rhs=ap3(c64[0:1, 0:1], 512, 2, 1, take),
                            start=first, stop=False, perf_mode=DR)
                        first = False
                        c += take

            def emit_av(h, g, aT, avt):
                for (kind, pe, oc, w, qp0, cb) in lay["av_pieces"][g]:
                    if kind == "vst":
                        rhs = vst4[0:KEYS, pe, h, 0:65]
                    else:
                        rhs = vste4[0:KEYS, pe, h, 0:65]
                    nc.tensor.matmul(
                        avt[qp0:qp0 + w, cb:cb + 65],
                        lhsT=aT[0:KEYS, oc:oc + w], rhs=rhs,
                        start=False, stop=False)

            def emit_head_tail(h, avt):
                # 1-col closers: end each bank's group on all 128 partitions
                for (b0, bw) in av_banks:
                    nc.tensor.matmul(avt[:, b0:b0 + 1],
                                     lhsT=ap3(c64[0:1, 0:1], 512, 2, 1, 128),
                                     rhs=ap3(c64[0:1, 0:1], 512, 2, 1, 1),
                                     start=False, stop=True, perf_mode=DR)
                rcol = nrm.tile([128, 17], f32, tag="rcol", name="rcol")
                for (i0, i1, base, n) in ((0, 7, 64, 7),
                                          (7, 14, 512 + 64, 7),
                                          (14, 17, 1024 + 64, 3)):
                    d0 = avt[:, base:base + 1]
                    nc.vector.reciprocal(
                        rcol[:, i0:i1],
                        bass.AP(tensor=d0.tensor, offset=d0.offset,
                                ap=[list(d0.ap[0]), [65, n]]))
                for s in range(NSLOT):
                    eng = nc.vector if s % 2 == 0 else nc.gpsimd
                    eng.tensor_scalar_mul(
                        outS[:, s * D + h * DH:s * D + h * DH + DH],
                        avt[:, slot_col(s):slot_col(s) + 64],
                        rcol[:, s:s + 1])
                nc.vector.tensor_scalar_mul(
                    outS[0:1, NSLOT * D + h * DH:NSLOT * D + h * DH + DH],
                    avt[0:1, CLSB:CLSB + 64], rcol[0:1, 16:17])

            SKEW = int(os.environ.get("BEIT_SKEW", "2"))
            pending = []
            avt_by_h = {}
            for h in range(NH):
                dt, r0 = h // 2, 64 * (h % 2)
                avt = avps.tile([128, 1536], f32, tag="avt", name="avt")
                avt_by_h[h] = avt
                for g in range(ng):
                    gw = lay["gocc"][g]
                    sc = scps.tile([128, 1024], f32, tag="sc", name="sc")
                    lb = lbp.tile([50, 2048], f8, tag="lb", name="lb")
                    nc.sync.dma_start(out=lb[:, :], in_=lb8_d[h, g])
                    for (p, rc, rw, oc, st) in lay["score_pieces"][g]:
                        if QK8:
                            lhsT = ap3(kT8[r0:r0 + 64, dt * SP8 + 99 * p:dt * SP8 + 99 * p + 1],
                                       KZ - 99 * p, 2, 1, KEYS)
                            rhs = ap3(qT8[r0:r0 + 64, dt * SP8 + rc:dt * SP8 + rc + 1],
                                      QZ - rc, 2, 1, rw)
                            nc.tensor.matmul(sc[0:KEYS, oc:oc + rw], lhsT=lhsT,
                                             rhs=rhs, start=st, stop=False,
                                             perf_mode=DR)
                        else:
                            nc.tensor.matmul(
                                sc[0:KEYS, oc:oc + rw],
                                lhsT=kT8[r0:r0 + 64, dt * SP8 + 99 * p:dt * SP8 + 99 * p + KEYS],
                                rhs=qT8[r0:r0 + 64, dt * SP8 + rc:dt * SP8 + rc + rw],
                                start=st, stop=False)
                    for (bc0, bw, sp) in lay["bias_pieces"][g]:
                        lhsT = ap3(i8sb[0:50, 0:1], KEYS, 2, 1, KEYS)
                        rhs = ap3(lb[0:50, bc0:bc0 + 1], 1024, 2, 1, bw)
                        nc.tensor.matmul(sc[0:KEYS, bc0:bc0 + bw], lhsT=lhsT,
                                         rhs=rhs, start=False, stop=sp,
                                         perf_mode=DR)
                    aT = ab.tile([KEYS, 1024], bf, tag="aT", name="aT")
                    nc.scalar.activation(aT[:, 0:gw], sc[0:KEYS, 0:gw], Exp)
                    if DEBUG and h == 0 and g == 0:
                        nc.sync.dma_start(out=dbga_d[:, 0:gw], in_=aT[0:KEYS, 0:gw])
                    pending.append((h, g, aT))
                    if len(pending) > SKEW:
                        ph, pg, paT = pending.pop(0)
                        if pg == 0:
                            emit_openers(avt_by_h[ph])
                        emit_av(ph, pg, paT, avt_by_h[ph])
                        if pg == ng - 1:
                            emit_head_tail(ph, avt_by_h.pop(ph))
            for (ph, pg, paT) in pending:
                if pg == 0:
                    emit_openers(avt_by_h[ph])
                emit_av(ph, pg, paT, avt_by_h[ph])
                if pg == ng - 1:
                    emit_head_tail(ph, avt_by_h.pop(ph))

            if DEBUG:
                nc.sync.dma_start(out=dbgq_d[:, :], in_=qT8[:, :])
                nc.sync.dma_start(out=dbgk_d[:, :], in_=kT8[:, :])
                nc.sync.dma_start(out=dbgv_d[:, :], in_=vst[:, :])
            # output DMA per slot half (+ cls token row)
            for s in range(NSLOT):
                t0 = 1 + 98 * s
                nc.gpsimd.dma_start(out=out_d[t0:t0 + 49, :],
                                    in_=outS[0:49, s * D:(s + 1) * D])
                nc.gpsimd.dma_start(out=out_d[t0 + 49:t0 + 98, :],
                                    in_=outS[64:113, s * D:(s + 1) * D])
            nc.gpsimd.dma_start(out=out_d[0:1, :],
                                in_=outS[0:1, NSLOT * D:(NSLOT + 1) * D])

    _split_excess_waits(nc, mybir, limit=1)
    return nc


# ----------------------------------------------------------------------------
# host-side input prep
# ----------------------------------------------------------------------------

def _prepare(hidden_states, Wq, bq, Wk, Wv, bv, rel_table, rel_pos_index, rand_idx):
    import ml_dtypes

    import concourse.bass as bass
    import concourse.tile as tile
    from concourse import mybir

    _patch_tile_drain()
    f8np = ml_dtypes.float8_e4m3

    hidden_states = np.asarray(hidden_states, np.float32)
    Wq = np.asarray(Wq, np.float32)
    Wk = np.asarray(Wk, np.float32)
    Wv = np.asarray(Wv, np.float32)
    bq = np.asarray(bq, np.float32)
    bv = np.asarray(bv, np.float32)
    rel_table = np.asarray(rel_table, np.float32)
    rel_pos_index = np.asarray(rel_pos_index)
    rand_idx = np.asarray(rand_idx)

    lay = _build_layout(rand_idx)
    lb8 = _build_bias8(lay, rel_table, rel_pos_index, f8np)
    f16np = np.float16

    i8 = np.zeros((50, 2, KEYS), np.float32)
    for p in range(49):
        i8[p, 0, p] = 1.0 / 16.0
        i8[p, 1, 49 + p] = 1.0 / 16.0
    i8[49, 1, 98] = 1.0 / 16.0

    shared = {
        "Wq16": Wq.astype(f16np), "Wk16": Wk.astype(f16np),
        "Wv16": Wv.astype(f16np),
        "bq8": np.concatenate([bq * 64.0, np.zeros(256, np.float32)]
                              ).reshape(1, 1024).astype(f8np),
        "bv16": bv.reshape(1, D).astype(f16np),
        "I8": i8.reshape(50, 2 * KEYS).astype(f8np),
        "lb8": lb8,
        "cz16": np.concatenate([np.zeros(1024, np.float32),
                                np.ones(1024, np.float32)]
                               ).reshape(1, 2048).astype(ml_dtypes.bfloat16),
    }

    # pair-major token order for the k/v projections (cls duplicated per pair)
    korder = np.empty(KCOLS, np.int64)
    for p in range(NPAIR):
        korder[99 * p:99 * p + 98] = 1 + 98 * p + np.arange(98)
        korder[99 * p + 98] = 0

    in_maps = []
    for b in range(B):
        hsT = hidden_states[b].T  # [768, S]
        m = dict(shared)
        m["hs16"] = np.ascontiguousarray(hsT).astype(f16np)
        m["hsk16"] = np.ascontiguousarray(hsT[:, korder]).astype(f16np)
        in_maps.append(m)

    nc = bass.Bass()
    _emit(nc, tile, mybir, lay)
    return nc, in_maps


# ----------------------------------------------------------------------------
# optional PJRT repeat-bench (unused by default; kept from v1)
# ----------------------------------------------------------------------------

def _bench_pjrt(nc, in_maps, n_cores, iters=20, warmup=3):
    import time

    import jax
    from jax.sharding import Mesh, PartitionSpec
    from jax.experimental.shard_map import shard_map

    from concourse import mybir
    from concourse.bass2jax import (_bass_exec_p, install_neuronx_cc_hook,
                                    partition_id_tensor)

    install_neuronx_cc_hook()
    partition_name = nc.partition_id_tensor.name if nc.partition_id_tensor else None
    in_names, out_names, out_avals, zero_outs = [], [], [], []
    for alloc in nc.m.functions[0].allocations:
        if not isinstance(alloc, mybir.MemoryLocationSet):
            continue
        name = alloc.memorylocations[0].name
        if alloc.kind == "ExternalInput":
            if name != partition_name:
                in_names.append(name)
        elif alloc.kind == "ExternalOutput":
            shape = tuple(alloc.tensor_shape)
            dtype = mybir.dt.np(alloc.dtype)
            out_names.append(name)
            out_avals.append(jax.core.ShapedArray(shape, dtype))
            zero_outs.append(np.zeros(shape, dtype))
    n_params = len(in_names)
    all_in_names = in_names + out_names + ([partition_name] if partition_name else [])

    def _body(*args):
        operands = list(args)
        if partition_name is not None:
            operands.append(partition_id_tensor())
        return tuple(_bass_exec_p.bind(
            *operands,
            out_avals=tuple(out_avals),
            in_names=tuple(all_in_names),
            out_names=tuple(out_names),
            lowering_input_output_aliases=(),
            sim_require_finite=True,
            sim_require_nnan=True,
            nc=nc,
        ))

    devices = jax.devices()[:n_cores]
    mesh = Mesh(np.asarray(devices), ("core",))
    n_outs = len(out_names)
    sharded = jax.jit(
        shard_map(_body, mesh=mesh,
                  in_specs=(PartitionSpec("core"),) * (n_params + n_outs),
                  out_specs=(PartitionSpec("core"),) * n_outs,
                  check_rep=False),
        keep_unused=True,
    )
    per_core = [[np.asarray(m[name]) for name in in_names] for m in in_maps]
    concat_in = [np.concatenate([per_core[c][i] for c in range(n_cores)], axis=0)
                 for i in range(n_params)]
    concat_zeros = [np.zeros((n_cores * z.shape[0], *z.shape[1:]), z.dtype)
                    for z in zero_outs]
    dev_in = [jax.device_put(a) for a in concat_in + concat_zeros]
    out = sharded(*dev_in)
    jax.block_until_ready(out)
    for _ in range(warmup):
        out = sharded(*dev_in)
    jax.block_until_ready(out)
    t0 = time.perf_counter()
    for _ in range(iters):
        out = sharded(*dev_in)
    jax.block_until_ready(out)
    dt = (time.perf_counter() - t0) / iters
    results = [
        {name: np.asarray(out[i]).reshape(n_cores, *out_avals[i].shape)[c]
         for i, name in enumerate(out_names)}
        for c in range(n_cores)
    ]
    return int(dt * 1e9), results


# ----------------------------------------------------------------------------
# public entry point
# ----------------------------------------------------------------------------

def kernel(hidden_states, Wq, bq, Wk, Wv, bv, rel_table, rel_pos_index, rand_idx):
    from concourse.bass_utils import run_bass_kernel_spmd

    nc, in_maps = _prepare(hidden_states, Wq, bq, Wk, Wv, bv,
                           rel_table, rel_pos_index, rand_idx)

    kernel.last_nc = nc
    kernel.last_in_maps = in_maps
    bench_iters = int(os.environ.get("BEIT_BENCH", "0"))
    if bench_iters > 0:
        per_iter_ns, results = _bench_pjrt(nc, in_maps, N_CORES, iters=bench_iters)
        kernel.last_exec_time_ns = per_iter_ns
    else:
        res = run_bass_kernel_spmd(nc, in_maps, core_ids=list(range(N_CORES)))
        results = res.results

    out = np.empty((B, S, D), np.float32)
    for b in range(B):
        out[b] = results[b]["out_s"]
    return out


# revision 33
# speedup vs baseline: 1.1032x; 1.0800x over previous
"""BeitSelfAttention block-sparse attention kernel for 8 Trainium2 NeuronCores.

Strategy (data-parallel over batch, B=8 -> one batch element per core):
  - fp8e4 DoubleRow matmuls for QKV projections and block-sparse scores
    (two 128-row k-tiles per pass at 0.5 cycles/row).
  - The relative-position bias AND the block-sparsity mask are host-packed
    into one fp8 table (16x scale) and added into the score PSUM by an
    identity DoubleRow matmul; gather multiplicity (rand/local block
    collisions) is realized by a few extra AV matmuls against half-masked V
    copies, so no per-element multiply is needed on DVE at all.
  - The cls KEY rides as a 99th score row per key-pair tile (designated to
    one pair per query via the bias mask); the cls QUERY is packed column 0.
  - AV is computed transposed: out[q, dh] psum tiles [128 queries, 65] with a
    ones-rider column in V accumulating the softmax denominator per query
    IN THE FREE DIM, so normalization is a per-partition reciprocal +
    scaled copy (no cross-partition broadcast, no DRAM round trip).
  - Output written as [S, 768] fp32 - already the final layout.
"""

import os
from contextlib import ExitStack

import numpy as np

NCLS, BS, NBLK, NPAIR, NH, DH = 1, 49, 32, 16, 12, 64
B, S, D = 8, 1569, 768
NTOK = S - NCLS  # 1568
N_CORES = 8
QK8 = int(__import__('os').environ.get('QK8', '1')) != 0
KEYS = 99            # 98 pair keys + 1 cls row
NSLOT = 16           # AV psum slots (2 blocks each, partitions 0-48 / 64-112)
CLSB = 1024 + 2 * 65  # cls-query corner col in the AV psum tile (bank 2)
AVW = CLSB + 65      # used width of the AV psum tile
SP8 = 1856           # per-dt stride of qT8/kT8 (zero strip beyond data)
KCOLS = NPAIR * KEYS  # 1584 pair-major key columns
QZ = 1576            # qT8 ktile1 landing col (zeros)
KZ = 1584            # kT8 ktile1 landing col (zeros)


# ----------------------------------------------------------------------------
# host-side layout
# ----------------------------------------------------------------------------

def _slot_target(qtok):
    """Map a query token to its AV psum target: (partition0, col_base).
    Slot s holds block 2s at partitions 0-48 and block 2s+1 at 64-112;
    the cls query (token 0) lives at partition 0 of the cls corner."""
    if qtok == 0:
        return 0, CLSB
    qb = (qtok - 1) // BS
    s = qb // 2
    cb = 65 * s if s < 7 else (512 + 65 * (s - 7) if s < 14 else 1024 + 65 * (s - 14))
    return 64 * (qb % 2) + (qtok - 1 - BS * qb), cb


def _build_layout(rand_idx):
    rand_idx = np.asarray(rand_idx)
    mult = np.zeros((NBLK, NBLK), np.int32)
    for m in range(NBLK):
        for o in (-1, 0, 1):
            mult[m, (m + o) % NBLK] += 1
        for r in rand_idx[m]:
            mult[m, int(r)] += 1

    # pack attending query columns per key-pair into banks of 512 (groups of
    # 1024). Block units (the cls col, or a 49-token block) never straddle a
    # bank boundary: the packing pads to the bank edge instead, so every AV
    # piece starts at an aligned psum partition (0 or 64). Pad columns carry
    # no scores; the bias matmul assigns them the -448 mask.
    segs = []
    gcol = 0
    for p in range(NPAIR):
        att = sorted(set(np.nonzero(mult[:, 2 * p])[0])
                     | set(np.nonzero(mult[:, 2 * p + 1])[0]))
        units = [(0, 1)] + [(1 + BS * m, BS) for m in att]
        cur = None
        prev_end = None
        for (uc, uw) in units:
            if 512 - (gcol % 512) < uw:
                gcol += 512 - (gcol % 512)  # pad to bank edge
                cur = None
            if cur is None or cur["bank"] != gcol // 512:
                cur = {"p": p, "runs": [], "bank": gcol // 512,
                       "off": gcol % 512}
                segs.append(cur)
                prev_end = None
            if prev_end == uc:
                rc0, rw0 = cur["runs"][-1]
                cur["runs"][-1] = (rc0, rw0 + uw)
            else:
                cur["runs"].append((uc, uw))
            prev_end = uc + uw
            gcol += uw
        cur = None  # next pair starts a new segment

    nbank = (gcol + 511) // 512
    ng = (nbank + 1) // 2
    for sg in segs:
        sg["g"] = sg["bank"] // 2
        sg["goff"] = (sg["bank"] % 2) * 512 + sg["off"]
    # group occupancy (incl. pad columns): all banks full except the last
    last_bank_fill = gcol - (nbank - 1) * 512
    gocc = []
    for g in range(ng):
        b0, b1 = 2 * g, 2 * g + 1
        occ = 0
        for b in (b0, b1):
            if b < nbank - 1:
                occ += 512
            elif b == nbank - 1:
                occ += last_bank_fill
        gocc.append(occ)

    # per-group score pieces (runs split to <=256), start flag per 512-region
    score_pieces = [[] for _ in range(ng)]
    bank_started = [False] * nbank
    for sg in segs:
        oc = 0
        for (rc, rw) in sg["runs"]:
            c, w = rc, rw
            while w > 0:
                take = min(w, 256)
                st = not bank_started[sg["bank"]]
                bank_started[sg["bank"]] = True
                score_pieces[sg["g"]].append(
                    (sg["p"], c, take, sg["goff"] + oc, st))
                oc += take
                c += take
                w -= take

    # bias pieces per group: cover each bank's occupancy in <=256 chunks,
    # split at the pad watermark (scores wrote [0, used); pads [used, bw) are
    # still pending-zero and must be covered by their own assign piece);
    # last chunk per bank carries stop
    bank_used = [0] * nbank
    for sg in segs:
        w = sum(rw for (_, rw) in sg["runs"])
        bank_used[sg["bank"]] = max(bank_used[sg["bank"]], sg["off"] + w)
    bias_pieces = [[] for _ in range(ng)]
    for g in range(ng):
        for half in range(2):
            b = 2 * g + half
            if b >= nbank:
                continue
            bw = 512 if b < nbank - 1 else last_bank_fill
            used = bank_used[b]
            c = 0
            while c < bw:
                lim = used if c < used else bw
                take = min(256, lim - c)
                bias_pieces[g].append(
                    (half * 512 + c, take, c + take >= bw))
                c += take

    # AV pieces per group: runs split at block units -> aligned partitions
    av_pieces = [[] for _ in range(ng)]
    for sg in segs:
        oc = 0
        for (rc, rw) in sg["runs"]:
            c, w = rc, rw
            while w > 0:
                take = 1 if c == 0 else min(w, BS - (c - 1) % BS)
                qp0, cb = _slot_target(c)
                av_pieces[sg["g"]].append(
                    ("vst", sg["p"], sg["goff"] + oc, take, qp0, cb))
                oc += take
                c += take
                w -= take

    # per-group column -> (qtok, pair) maps (qtok -1 = pad), cls designation
    lb_cols = []
    for g in range(ng):
        qtok = np.full(1024, -1, np.int64)
        pair = np.zeros(1024, np.int64)
        lb_cols.append((qtok, pair))
    cls_seen = np.zeros(S, bool)
    cls_des = np.zeros((ng, 1024), bool)
    for sg in segs:
        qtok, pair = lb_cols[sg["g"]]
        oc = sg["goff"]
        for (rc, rw) in sg["runs"]:
            qtok[oc:oc + rw] = np.arange(rc, rc + rw)
            pair[oc:oc + rw] = sg["p"]
            fresh = ~cls_seen[rc:rc + rw]
            cls_des[sg["g"], oc:oc + rw] = fresh
            cls_seen[rc:rc + rw] = True
            oc += rw

    # multiplicity extras: (qb, kb) with mult >= 2 -> (m-1) extra AV matmuls
    # against a half-masked V copy (vste slot per distinct (pair, half))
    vste_cases = []      # (pair, half)
    col_of = {}
    for sg in segs:
        oc = sg["goff"]
        for (rc, rw) in sg["runs"]:
            for i in range(rw):
                col_of[(sg["p"], rc + i)] = (sg["g"], oc + i)
            oc += rw
    for qb in range(NBLK):
        for kb in range(NBLK):
            m = int(mult[qb, kb])
            if m < 2:
                continue
            p, half = kb // 2, kb % 2
            if (p, half) not in vste_cases:
                vste_cases.append((p, half))
            e = vste_cases.index((p, half))
            t0 = 1 + BS * qb
            g, oc = col_of[(p, t0)]
            qp0, cb = _slot_target(t0)
            for _ in range(m - 1):
                av_pieces[g].append(("vste", e, oc, BS, qp0, cb))
    return {"segs": segs, "mult": mult, "ng": ng, "gocc": gocc,
            "nbank": nbank, "score_pieces": score_pieces,
            "bias_pieces": bias_pieces, "av_pieces": av_pieces,
            "lb_cols": lb_cols, "cls_des": cls_des,
            "vste_cases": vste_cases}


def _build_bias8(lay, rel_table, rel_pos_index, f8np):
    """lb8 [NH, ng, 50, 2048]: rows (p, i) -> key 49i+p (cls at (49,1)),
    values 16*bias, -240 where masked."""
    ng = lay["ng"]
    mult = lay["mult"]
    MASK = -240.0
    lb = np.full((NH, ng, 50, 2, 1024), MASK, np.float32)
    for sg in lay["segs"]:
        g = sg["g"]
        p = sg["p"]
        ktok = 1 + 98 * p + np.arange(98)          # [98]
        kblk = 2 * p + (np.arange(98) // BS)
        oc = sg["goff"]
        for (rc, rw) in sg["runs"]:
            qtok = np.arange(rc, rc + rw)
            qblk = np.maximum(qtok - 1, 0) // BS
            att = (mult[qblk][:, kblk] > 0) | (qtok == 0)[:, None]  # [rw, 98]
            idx = rel_pos_index[qtok[:, None], ktok[None, :]]       # [rw, 98]
            val = 16.0 * rel_table[idx]                             # [rw,98,NH]
            val = np.where(att[:, :, None], np.clip(val, -200, 200), MASK)
            v = val.transpose(2, 1, 0)                              # [NH,98,rw]
            lb[:, g, 0:49, 0, oc:oc + rw] = v[:, 0:49]
            lb[:, g, 0:49, 1, oc:oc + rw] = v[:, 49:98]
            # cls row: designated pair only
            des = lay["cls_des"][g, oc:oc + rw]
            cidx = rel_pos_index[qtok, 0]
            cval = np.clip(16.0 * rel_table[cidx], -200, 200)       # [rw, NH]
            cv = np.where(des[:, None], cval, MASK).T               # [NH, rw]
            lb[:, g, 49, 1, oc:oc + rw] = cv
            oc += rw
    return lb.reshape(NH, ng, 50, 2048).astype(f8np)


# ----------------------------------------------------------------------------
# walrus workaround: split the TileContext tail drain's sem waits
# ----------------------------------------------------------------------------

def _patch_tile_drain():
    import concourse.tile as tile
    from concourse.vector_clock import ScopedClock, VectorClock

    if getattr(tile.TileContext, "_beit_drain_patch", False):
        return

    def _drain_and_barrier(self, tick_clock, wait_clock):
        gc_vec = tick_clock.global_clock
        n = len(gc_vec)
        nonzero = [i for i in range(n) if gc_vec[i] > 0] or [0]
        for i in range(0, len(nonzero), 1):
            chunk = set(nonzero[i:i + 1])
            vec = VectorClock([gc_vec[j] if j in chunk else 0 for j in range(n)])
            drain_inst = self.nc.sync.drain()
            wait_clock.add_sem_waits(drain_inst.ins, ScopedClock({None: vec}))
        self.nc.all_engine_barrier()
        assert self.sems is not None
        popped = self.nc._tile_sem_poison_stack.pop()
        assert popped is self._sem_poison
        self.nc.clear_and_free_semaphores(list(self.sems.allocated().values()))
        self.nc.all_engine_barrier()

    tile.TileContext._drain_and_barrier = _drain_and_barrier
    tile.TileContext._beit_drain_patch = True


def _split_excess_waits(nc, mybir, limit=1):
    """This walrus build allows very few sem waits per instruction; move the
    excess onto EventSemaphore carrier instructions inserted just before."""
    ctr = [0]
    for f in nc.m.functions:
        for bb in f.blocks:
            il = bb.instructions
            out = []
            for inst in il:
                si = inst.sync_info
                if si is not None and si.on_wait and len(si.on_wait) > limit:
                    waits = list(si.on_wait)
                    over = waits[limit:]
                    for j in range(0, len(over), limit):
                        ctr[0] += 1
                        ev = mybir.InstEventSemaphore(
                            name=f"WSPLIT-{ctr[0]}", ins=[], outs=[],
                            engine=inst.engine,
                            sync_info=mybir.SyncInfo(on_wait=over[j:j + limit],
                                                     on_update=[]),
                        )
                        nc.register_instruction(ev, overwrite=True)
                        out.append(ev)
                    si.on_wait = waits[:limit]
                out.append(inst)
            il[:] = out
    return ctr[0]


# ----------------------------------------------------------------------------
# device kernel emission
# ----------------------------------------------------------------------------

def _emit(nc, tile, mybir, lay):
    import concourse.bass as bass

    bf = mybir.dt.bfloat16
    f8 = mybir.dt.float8e4
    f32 = mybir.dt.float32
    DR = mybir.MatmulPerfMode.DoubleRow
    Exp = mybir.ActivationFunctionType.Exp
    ng = lay["ng"]
    NE = max(1, len(lay["vste_cases"]))

    f16 = mybir.dt.float16
    hs16_d = nc.dram_tensor("hs16", [D, S], f16, kind="ExternalInput")
    hsk16_d = nc.dram_tensor("hsk16", [D, KCOLS], f16, kind="ExternalInput")
    wq16_d = nc.dram_tensor("Wq16", [D, D], f16, kind="ExternalInput")
    wk16_d = nc.dram_tensor("Wk16", [D, D], f16, kind="ExternalInput")
    wv16_d = nc.dram_tensor("Wv16", [D, D], f16, kind="ExternalInput")
    bq8_d = nc.dram_tensor("bq8", [1, 1024], f8, kind="ExternalInput")
    bv16_d = nc.dram_tensor("bv16", [1, D], f16, kind="ExternalInput")
    i8_d = nc.dram_tensor("I8", [50, 2 * KEYS], f8, kind="ExternalInput")
    cz16_d = nc.dram_tensor("cz16", [1, 2048], bf, kind="ExternalInput")
    lb8_d = nc.dram_tensor("lb8", [NH, ng, 50, 2048], f8, kind="ExternalInput")
    out_d = nc.dram_tensor("out_s", [S, D], f32, kind="ExternalOutput")
    DEBUG = int(os.environ.get("BEIT_DEBUG", "0"))
    if DEBUG:
        dbgq_d = nc.dram_tensor("dbg_q", [128, 6 * SP8], f8 if QK8 else bf, kind="ExternalOutput")
        dbgk_d = nc.dram_tensor("dbg_k", [128, 6 * SP8], f8 if QK8 else bf, kind="ExternalOutput")
        dbga_d = nc.dram_tensor("dbg_a", [KEYS, 1024], bf, kind="ExternalOutput")
        dbgv_d = nc.dram_tensor("dbg_v", [KEYS, NPAIR * NH * 65], bf, kind="ExternalOutput")

    def ap3(sl, s1, n1, s2, n2):
        return bass.AP(tensor=sl.tensor, offset=sl.offset,
                       ap=[list(sl.ap[0]), [s1, n1], [s2, n2]])

    def slot_col(s):
        return 65 * s if s < 7 else (512 + 65 * (s - 7) if s < 14
                                     else 1024 + 65 * (s - 14))

    av_banks = [(0, 455), (512, 455), (1024, AVW - 1024)]

    with tile.TileContext(nc) as tc, ExitStack() as ctx:
        consts = ctx.enter_context(tc.tile_pool(name="consts", bufs=1))
        persist = ctx.enter_context(tc.tile_pool(name="persist", bufs=1))

        c64 = consts.tile([1, 1024], f8, tag="c64", name="c64")
        nc.vector.memset(c64[:, :], 1.0 / 64.0)
        o8c = consts.tile([1, 1024], f8, tag="o8c", name="o8c")
        nc.vector.memset(o8c[:, 0:512], 1.0 / 64.0)
        nc.vector.memset(o8c[:, 512:1024], 0.0)
        bq8 = consts.tile([1, 1024], f8, tag="bq8", name="bq8")
        nc.gpsimd.dma_start(out=bq8[:, :], in_=bq8_d[:, :])
        bv16 = consts.tile([1, D], f16, tag="bv16", name="bv16")
        nc.gpsimd.dma_start(out=bv16[:, :], in_=bv16_d[:, :])
        ones16 = consts.tile([1, 128], f16, tag="ones16", name="ones16")
        nc.vector.memset(ones16[:, :], 1.0)
        i8sb = consts.tile([50, 2 * KEYS], f8, tag="i8", name="i8")
        nc.gpsimd.dma_start(out=i8sb[:, :], in_=i8_d[:, :])

        qkdt = f8 if QK8 else bf
        qT8 = persist.tile([128, 6 * SP8], qkdt, tag="qT8", name="qT8")
        kT8 = persist.tile([128, 6 * SP8], qkdt, tag="kT8", name="kT8")
        for t in range(6):
            nc.vector.memset(qT8[:, t * SP8 + S:(t + 1) * SP8], 0.0)
            nc.vector.memset(kT8[:, t * SP8 + KCOLS:(t + 1) * SP8], 0.0)
        vst = persist.tile([KEYS, NPAIR * NH * 65], bf, tag="vst", name="vst")
        vst4 = vst[:, :].rearrange("a (p h e) -> a p h e", p=NPAIR, h=NH)
        nc.vector.memset(vst4[:, :, :, 64:65], 1.0)
        vste = persist.tile([KEYS, NE * NH * 65], bf, tag="vste", name="vste")
        nc.gpsimd.memset(vste[:, :], 0.0)
        vste4 = vste[:, :].rearrange("a (e h c) -> a e h c", e=NE, h=NH)
        def bcast49(dram_sl, inner):
            # DRAM source broadcast across 49 partitions
            return bass.AP(tensor=dram_sl.tensor, offset=dram_sl.offset,
                           ap=[[0, 49]] + inner)
        outS = persist.tile([128, (NSLOT + 1) * D], f32, tag="outS", name="outS")

        # ---------------- phase A: projections ----------------
        with tc.tile_pool(name="phA", bufs=1) as phA, \
             tc.tile_pool(name="pp", bufs=3, space="PSUM") as pp, \
             tc.tile_pool(name="ppv", bufs=2, space="PSUM") as ppv:
            hs16, hsk16 = [], []
            w_sb = {"q": [], "k": [], "v": []}
            for t in range(6):
                ht = phA.tile([128, S], f16, tag=f"hs16_{t}", name=f"hs16_{t}")
                nc.sync.dma_start(out=ht[:, :], in_=hs16_d[t * 128:(t + 1) * 128, :])
                hs16.append(ht)
                wt = phA.tile([128, D], f16, tag=f"wq16_{t}", name=f"wq16_{t}")
                nc.sync.dma_start(out=wt[:, :], in_=wq16_d[t * 128:(t + 1) * 128, :])
                w_sb["q"].append(wt)
            for t in range(6):
                ht = phA.tile([128, KCOLS], f16, tag=f"hsk16_{t}", name=f"hsk16_{t}")
                nc.gpsimd.dma_start(out=ht[:, :], in_=hsk16_d[t * 128:(t + 1) * 128, :])
                hsk16.append(ht)
            for nm, dram in (("k", wk16_d), ("v", wv16_d)):
                for t in range(6):
                    wt = phA.tile([128, D], f16, tag=f"w{nm}16_{t}", name=f"w{nm}16_{t}")
                    nc.gpsimd.dma_start(out=wt[:, :],
                                        in_=dram[t * 128:(t + 1) * 128, :])
                    w_sb[nm].append(wt)

            # qT8 / kT8 projections (fp16 matmuls, fp8 store): psum [128, chunk]
            qchunks = [(0, 512), (512, 512), (1024, 512), (1536, S - 1536)]
            kchunks = [(0, 512), (512, 512), (1024, 512), (1536, KCOLS - 1536)]
            for name, hsrc, chunks, dst, scale in (
                    ("q", hs16, qchunks, qT8, 0.5),
                    ("k", hsk16, kchunks, kT8, 0.25)):
                for dt in range(6):
                    for (c0, cw) in chunks:
                        ps = pp.tile([128, 512], f32, tag="pq", name="pq")
                        for kt in range(6):
                            nc.tensor.matmul(
                                ps[:, 0:cw],
                                lhsT=w_sb[name][kt][:, dt * 128:(dt + 1) * 128],
                                rhs=hsrc[kt][:, c0:c0 + cw],
                                start=(kt == 0),
                                stop=(kt == 5 and name == "k"))
                        if name == "q":
                            s0 = 0
                            while s0 < cw:
                                sw = min(256, cw - s0)
                                lhsT = ap3(bq8[0:1, dt * 128:dt * 128 + 1],
                                           D - dt * 128, 2, 1, 128)
                                rhs = ap3(c64[0:1, 0:1], 512, 2, 1, sw)
                                nc.tensor.matmul(ps[:, s0:s0 + sw], lhsT=lhsT,
                                                 rhs=rhs, start=False,
                                                 stop=(s0 + sw >= cw),
                                                 perf_mode=DR)
                                s0 += sw
                        nc.vector.tensor_scalar_mul(
                            dst[:, dt * SP8 + c0:dt * SP8 + c0 + cw],
                            ps[:, 0:cw], scale)

            # V projection per pair (pair-major hs incl. the cls-dup column,
            # so row 98 of each pair slice is v_cls): psum [99 tokens, 768]
            ecase = {pc: e for e, pc in enumerate(lay["vste_cases"])}
            for p in range(NPAIR):
                c0 = KEYS * p
                ps = ppv.tile([128, D], f32, tag="pv", name="pv")
                for (h0, hw_) in ((0, 512), (512, 256)):
                    for kt in range(6):
                        nc.tensor.matmul(
                            ps[0:KEYS, h0:h0 + hw_],
                            lhsT=hsk16[kt][:, c0:c0 + KEYS],
                            rhs=w_sb["v"][kt][:, h0:h0 + hw_],
                            start=(kt == 0), stop=False)
                    nc.tensor.matmul(ps[0:KEYS, h0:h0 + hw_],
                                     lhsT=ones16[0:1, 0:KEYS],
                                     rhs=bv16[0:1, h0:h0 + hw_],
                                     start=False, stop=True)
                src = ps[0:KEYS, :].rearrange("a (h e) -> a h e", h=NH)
                nc.gpsimd.tensor_copy(vst4[0:KEYS, p, :, 0:64], src)
                for half in range(2):
                    if (p, half) not in ecase:
                        continue
                    e = ecase[(p, half)]
                    nc.gpsimd.tensor_copy(vste4[0:98, e, :, 0:64],
                                          ps[0:98, :].rearrange(
                                              "a (h e) -> a h e", h=NH))
                    if half == 0:
                        # zero the inactive upper half, rider=1 on lower
                        nc.sync.dma_start(out=vste4[49:98, e, :, 0:65],
                                          in_=bcast49(cz16_d[0:1, 0:1],
                                                      [[65, NH], [1, 65]]))
                        nc.gpsimd.memset(vste4[0:49, e, :, 64:65], 1.0)
                    else:
                        # zero the inactive lower half, rider=1 on upper
                        nc.gpsimd.memset(vste4[0:49, e, :, 0:65], 0.0)
                        nc.sync.dma_start(
                            out=vste4[49:98, e, :, 64:65],
                            in_=bcast49(cz16_d[0:1, 1024:1025],
                                        [[1, NH], [1, 1]]))

        # ---------------- phase B: block-sparse attention per head ----------
        with tc.tile_pool(name="scps", bufs=2, space="PSUM") as scps, \
             tc.tile_pool(name="avps", bufs=1, space="PSUM") as avps, \
             tc.tile_pool(name="ab", bufs=4) as ab, \
             tc.tile_pool(name="lbp", bufs=4) as lbp, \
             tc.tile_pool(name="nrm", bufs=2) as nrm:

            def emit_openers(avt):
                # init every used AV psum byte to a tiny value ((1/64)^2 * 2)
                for (b0, bw) in av_banks:
                    first = True
                    c = 0
                    while c < bw:
                        take = min(256, bw - c)
                        nc.tensor.matmul(
                            avt[:, b0 + c:b0 + c + take],
                            lhsT=ap3(c64[0:1, 0:1], 512, 2, 1, 128),
                            rhs=ap3(c64[0:1, 0:1], 512, 2, 1, take),
                            start=first, stop=False, perf_mode=DR)
                        first = False
                        c += take

            def emit_av(h, g, aT, avt):
                for (kind, pe, oc, w, qp0, cb) in lay["av_pieces"][g]:
                    if kind == "vst":
                        rhs = vst4[0:KEYS, pe, h, 0:65]
                    else:
                        rhs = vste4[0:KEYS, pe, h, 0:65]
                    nc.tensor.matmul(
                        avt[qp0:qp0 + w, cb:cb + 65],
                        lhsT=aT[0:KEYS, oc:oc + w], rhs=rhs,
                        start=False, stop=False)

            def emit_head_tail(h, avt):
                # 1-col closers: end each bank's group on all 128 partitions
                for (b0, bw) in av_banks:
                    nc.tensor.matmul(avt[:, b0:b0 + 1],
                                     lhsT=ap3(c64[0:1, 0:1], 512, 2, 1, 128),
                                     rhs=ap3(c64[0:1, 0:1], 512, 2, 1, 1),
                                     start=False, stop=True, perf_mode=DR)
                rcol = nrm.tile([128, 17], f32, tag="rcol", name="rcol")
                for (i0, i1, base, n) in ((0, 7, 64, 7),
                                          (7, 14, 512 + 64, 7),
                                          (14, 17, 1024 + 64, 3)):
                    d0 = avt[:, base:base + 1]
                    nc.vector.reciprocal(
                        rcol[:, i0:i1],
                        bass.AP(tensor=d0.tensor, offset=d0.offset,
                                ap=[list(d0.ap[0]), [65, n]]))
                # per-bank merged normalize: out[slot cols] = avt * recip
                # (reciprocal broadcast along each slot's 64 V columns)
                for (eng, s0, ns, ab0) in ((nc.vector, 0, 7, 0),
                                           (nc.gpsimd, 7, 7, 512),
                                           (nc.vector, 14, 2, 1024)):
                    src = avt[:, ab0:ab0 + 1]
                    rsl = rcol[:, s0:s0 + 1]
                    osl = outS[:, s0 * D + h * DH:s0 * D + h * DH + 1]
                    eng.tensor_mul(
                        bass.AP(tensor=osl.tensor, offset=osl.offset,
                                ap=[list(osl.ap[0]), [D, ns], [1, 64]]),
                        bass.AP(tensor=src.tensor, offset=src.offset,
                                ap=[list(src.ap[0]), [65, ns], [1, 64]]),
                        bass.AP(tensor=rsl.tensor, offset=rsl.offset,
                                ap=[list(rsl.ap[0]), [1, ns], [0, 64]]))
                nc.vector.tensor_scalar_mul(
                    outS[0:1, NSLOT * D + h * DH:NSLOT * D + h * DH + DH],
                    avt[0:1, CLSB:CLSB + 64], rcol[0:1, 16:17])

            SKEW = int(os.environ.get("BEIT_SKEW", "2"))
            pending = []
            avt_by_h = {}
            for h in range(NH):
                dt, r0 = h // 2, 64 * (h % 2)
                avt = avps.tile([128, 1536], f32, tag="avt", name="avt")
                avt_by_h[h] = avt
                for g in range(ng):
                    gw = lay["gocc"][g]
                    sc = scps.tile([128, 1024], f32, tag="sc", name="sc")
                    lb = lbp.tile([50, 2048], f8, tag="lb", name="lb")
                    nc.sync.dma_start(out=lb[:, :], in_=lb8_d[h, g])
                    for (p, rc, rw, oc, st) in lay["score_pieces"][g]:
                        if QK8:
                            lhsT = ap3(kT8[r0:r0 + 64, dt * SP8 + 99 * p:dt * SP8 + 99 * p + 1],
                                       KZ - 99 * p, 2, 1, KEYS)
                            rhs = ap3(qT8[r0:r0 + 64, dt * SP8 + rc:dt * SP8 + rc + 1],
                                      QZ - rc, 2, 1, rw)
                            nc.tensor.matmul(sc[0:KEYS, oc:oc + rw], lhsT=lhsT,
                                             rhs=rhs, start=st, stop=False,
                                             perf_mode=DR)
                        else:
                            nc.tensor.matmul(
                                sc[0:KEYS, oc:oc + rw],
                                lhsT=kT8[r0:r0 + 64, dt * SP8 + 99 * p:dt * SP8 + 99 * p + KEYS],
                                rhs=qT8[r0:r0 + 64, dt * SP8 + rc:dt * SP8 + rc + rw],
                                start=st, stop=False)
                    for (bc0, bw, sp) in lay["bias_pieces"][g]:
                        lhsT = ap3(i8sb[0:50, 0:1], KEYS, 2, 1, KEYS)
                        rhs = ap3(lb[0:50, bc0:bc0 + 1], 1024, 2, 1, bw)
                        nc.tensor.matmul(sc[0:KEYS, bc0:bc0 + bw], lhsT=lhsT,
                                         rhs=rhs, start=False, stop=sp,
                                         perf_mode=DR)
                    aT = ab.tile([KEYS, 1024], bf, tag="aT", name="aT")
                    nc.scalar.activation(aT[:, 0:gw], sc[0:KEYS, 0:gw], Exp)
                    if DEBUG and h == 0 and g == 0:
                        nc.sync.dma_start(out=dbga_d[:, 0:gw], in_=aT[0:KEYS, 0:gw])
                    pending.append((h, g, aT))
                    if len(pending) > SKEW:
                        ph, pg, paT = pending.pop(0)
                        if pg == 0:
                            emit_openers(avt_by_h[ph])
                        emit_av(ph, pg, paT, avt_by_h[ph])
                        if pg == ng - 1:
                            emit_head_tail(ph, avt_by_h.pop(ph))
            for (ph, pg, paT) in pending:
                if pg == 0:
                    emit_openers(avt_by_h[ph])
                emit_av(ph, pg, paT, avt_by_h[ph])
                if pg == ng - 1:
                    emit_head_tail(ph, avt_by_h.pop(ph))

            if DEBUG:
                nc.sync.dma_start(out=dbgq_d[:, :], in_=qT8[:, :])
                nc.sync.dma_start(out=dbgk_d[:, :], in_=kT8[:, :])
                nc.sync.dma_start(out=dbgv_d[:, :], in_=vst[:, :])
            # output DMA per slot half (+ cls token row)
            for s in range(NSLOT):
                t0 = 1 + 98 * s
                nc.gpsimd.dma_start(out=out_d[t0:t0 + 49, :],
                                    in_=outS[0:49, s * D:(s + 1) * D])
                nc.gpsimd.dma_start(out=out_d[t0 + 49:t0 + 98, :],
                                    in_=outS[64:113, s * D:(s + 1) * D])
            nc.gpsimd.dma_start(out=out_d[0:1, :],
                                in_=outS[0:1, NSLOT * D:(NSLOT + 1) * D])

    _split_excess_waits(nc, mybir, limit=1)
    return nc


# ----------------------------------------------------------------------------
# host-side input prep
# ----------------------------------------------------------------------------

def _prepare(hidden_states, Wq, bq, Wk, Wv, bv, rel_table, rel_pos_index, rand_idx):
    import ml_dtypes

    import concourse.bass as bass
    import concourse.tile as tile
    from concourse import mybir

    _patch_tile_drain()
    f8np = ml_dtypes.float8_e4m3

    hidden_states = np.asarray(hidden_states, np.float32)
    Wq = np.asarray(Wq, np.float32)
    Wk = np.asarray(Wk, np.float32)
    Wv = np.asarray(Wv, np.float32)
    bq = np.asarray(bq, np.float32)
    bv = np.asarray(bv, np.float32)
    rel_table = np.asarray(rel_table, np.float32)
    rel_pos_index = np.asarray(rel_pos_index)
    rand_idx = np.asarray(rand_idx)

    lay = _build_layout(rand_idx)
    lb8 = _build_bias8(lay, rel_table, rel_pos_index, f8np)
    f16np = np.float16

    i8 = np.zeros((50, 2, KEYS), np.float32)
    for p in range(49):
        i8[p, 0, p] = 1.0 / 16.0
        i8[p, 1, 49 + p] = 1.0 / 16.0
    i8[49, 1, 98] = 1.0 / 16.0

    shared = {
        "Wq16": Wq.astype(f16np), "Wk16": Wk.astype(f16np),
        "Wv16": Wv.astype(f16np),
        "bq8": np.concatenate([bq * 64.0, np.zeros(256, np.float32)]
                              ).reshape(1, 1024).astype(f8np),
        "bv16": bv.reshape(1, D).astype(f16np),
        "I8": i8.reshape(50, 2 * KEYS).astype(f8np),
        "lb8": lb8,
        "cz16": np.concatenate([np.zeros(1024, np.float32),
                                np.ones(1024, np.float32)]
                               ).reshape(1, 2048).astype(ml_dtypes.bfloat16),
    }

    # pair-major token order for the k/v projections (cls duplicated per pair)
    korder = np.empty(KCOLS, np.int64)
    for p in range(NPAIR):
        korder[99 * p:99 * p + 98] = 1 + 98 * p + np.arange(98)
        korder[99 * p + 98] = 0

    in_maps = []
    for b in range(B):
        hsT = hidden_states[b].T  # [768, S]
        m = dict(shared)
        m["hs16"] = np.ascontiguousarray(hsT).astype(f16np)
        m["hsk16"] = np.ascontiguousarray(hsT[:, korder]).astype(f16np)
        in_maps.append(m)

    nc = bass.Bass()
    _emit(nc, tile, mybir, lay)
    return nc, in_maps


# ----------------------------------------------------------------------------
# optional PJRT repeat-bench (unused by default; kept from v1)
# ----------------------------------------------------------------------------

def _bench_pjrt(nc, in_maps, n_cores, iters=20, warmup=3):
    import time

    import jax
    from jax.sharding import Mesh, PartitionSpec
    from jax.experimental.shard_map import shard_map

    from concourse import mybir
    from concourse.bass2jax import (_bass_exec_p, install_neuronx_cc_hook,
                                    partition_id_tensor)

    install_neuronx_cc_hook()
    partition_name = nc.partition_id_tensor.name if nc.partition_id_tensor else None
    in_names, out_names, out_avals, zero_outs = [], [], [], []
    for alloc in nc.m.functions[0].allocations:
        if not isinstance(alloc, mybir.MemoryLocationSet):
            continue
        name = alloc.memorylocations[0].name
        if alloc.kind == "ExternalInput":
            if name != partition_name:
                in_names.append(name)
        elif alloc.kind == "ExternalOutput":
            shape = tuple(alloc.tensor_shape)
            dtype = mybir.dt.np(alloc.dtype)
            out_names.append(name)
            out_avals.append(jax.core.ShapedArray(shape, dtype))
            zero_outs.append(np.zeros(shape, dtype))
    n_params = len(in_names)
    all_in_names = in_names + out_names + ([partition_name] if partition_name else [])

    def _body(*args):
        operands = list(args)
        if partition_name is not None:
            operands.append(partition_id_tensor())
        return tuple(_bass_exec_p.bind(
            *operands,
            out_avals=tuple(out_avals),
            in_names=tuple(all_in_names),
            out_names=tuple(out_names),
            lowering_input_output_aliases=(),
            sim_require_finite=True,
            sim_require_nnan=True,
            nc=nc,
        ))

    devices = jax.devices()[:n_cores]
    mesh = Mesh(np.asarray(devices), ("core",))
    n_outs = len(out_names)
    sharded = jax.jit(
        shard_map(_body, mesh=mesh,
                  in_specs=(PartitionSpec("core"),) * (n_params + n_outs),
                  out_specs=(PartitionSpec("core"),) * n_outs,
                  check_rep=False),
        keep_unused=True,
    )
    per_core = [[np.asarray(m[name]) for name in in_names] for m in in_maps]
    concat_in = [np.concatenate([per_core[c][i] for c in range(n_cores)], axis=0)
                 for i in range(n_params)]
    concat_zeros = [np.zeros((n_cores * z.shape[0], *z.shape[1:]), z.dtype)
                    for z in zero_outs]
    dev_in = [jax.device_put(a) for a in concat_in + concat_zeros]
    out = sharded(*dev_in)
    jax.block_until_ready(out)
    for _ in range(warmup):
        out = sharded(*dev_in)
    jax.block_until_ready(out)
    t0 = time.perf_counter()
    for _ in range(iters):
        out = sharded(*dev_in)
    jax.block_until_ready(out)
    dt = (time.perf_counter() - t0) / iters
    results = [
        {name: np.asarray(out[i]).reshape(n_cores, *out_avals[i].shape)[c]
         for i, name in enumerate(out_names)}
        for c in range(n_cores)
    ]
    return int(dt * 1e9), results


# ----------------------------------------------------------------------------
# public entry point
# ----------------------------------------------------------------------------

def kernel(hidden_states, Wq, bq, Wk, Wv, bv, rel_table, rel_pos_index, rand_idx):
    from concourse.bass_utils import run_bass_kernel_spmd

    nc, in_maps = _prepare(hidden_states, Wq, bq, Wk, Wv, bv,
                           rel_table, rel_pos_index, rand_idx)

    kernel.last_nc = nc
    kernel.last_in_maps = in_maps
    bench_iters = int(os.environ.get("BEIT_BENCH", "0"))
    if bench_iters > 0:
        per_iter_ns, results = _bench_pjrt(nc, in_maps, N_CORES, iters=bench_iters)
        kernel.last_exec_time_ns = per_iter_ns
    else:
        res = run_bass_kernel_spmd(nc, in_maps, core_ids=list(range(N_CORES)))
        results = res.results

    out = np.empty((B, S, D), np.float32)
    for b in range(B):
        out[b] = results[b]["out_s"]
    return out
